# revision 1
# baseline (speedup 1.0000x reference)
"""BigramAttn Trainium2 kernel (8-core SPMD, raw Bass).

Reference computation (per batch b):
  e[0]   = sum_k enc[0,k] * h[k]
  e[s]   = sum_k (enc[s-1,:] @ M)[k] * h[k] * enc[s,k]          (s >= 1)
  e[s]  += sum_{k<3} (h @ affect)[k] * emb[s,k]
  out    = softmax(e)                                            # over s

Sharding: data-parallel over batch B=32 across 8 cores (4 batches/core).

Per core (batch-major, transposed layout [H, S]; h pre-folded into per-batch
M_b = M @ diag(h_b) on the host):
  A_T[k, t]  = sum_j M_b[j,k] * encT[j, s0+t]        (PE fp32r, 16 MMs/step)
  P[k, t]    = A_T[k, t] * encT[k, s0+t+1]           (DVE tensor_tensor, x2)
  P[0:3,:]  += ha[k] * embT[k, t]                    (DVE stt, affect term)
  e[t]       = sum_kt ones^T @ P_kt                  (PE, 4 accumulating MMs)
  softmax over 4096 logits per batch, batched as [4, 4096].

Host pre-transposes the enc shard to [4, 512, 4096]: the PE contracts over
partitions so H must land on partitions; DMA-transpose is 2-byte-only on
trn2 and on-device transposing would double engine work. Bytes to HBM are
identical. All matmuls run float32r (full PE rate at N>=256; measured
end-to-end rel err ~2e-3). fp32r ISA rules: even moving-column counts,
8B-aligned dst at partition 0, fp32r-tagged producers end to end.

This walrus build accepts exactly ONE semaphore wait per instruction, so the
kernel is raw Bass: per-engine programs, counting semaphores, standalone
waits. DMA completions may reorder across transfers, so chunk DMAs chain on
per-lane semaphores (the scheme Tile's DMAHW lanes use).
"""

import functools

import numpy as np

import concourse.bass as bass
from concourse import mybir
from concourse.bass_utils import run_bass_kernel_spmd

S, B, H = 4096, 32, 512
NCORES = 8
BC = B // NCORES          # batches per core = 4
NK = H // 128             # h-chunks = 4
CH = 512                  # s-chunk width
CW = CH + 1               # chunk tile block width (1-col halo)
NCH = S // CH             # s-chunks per batch = 8
NBC = BC * NCH            # chunk-steps per core = 32
NSLOT = 4                 # enc chunk tiles (4 chunk-steps in flight)
NLANE = 4                 # DMA completion-ordering lanes
SETUP_DMAS = 2 + 4 + 1 + 4  # h_t, aff_p, 4 emb, ones, 4 slot col-seeds

F32 = mybir.dt.float32
F32R = mybir.dt.float32r


def slot(bc):
    return bc % NSLOT


@functools.lru_cache(maxsize=1)
def _build():
    nc = bass.Bass("TRN2", target_bir_lowering=False, debug=False)

    enc_t = nc.dram_tensor("enc_t", [BC, H, S], F32R, kind="ExternalInput").ap()
    m_hb = nc.dram_tensor("m_hb", [BC, H, H], F32R, kind="ExternalInput").ap()
    h_t = nc.dram_tensor("h_t", [128, NK * BC + 1], F32R,
                         kind="ExternalInput").ap()
    aff_p = nc.dram_tensor("aff_p", [128, NK * 3], F32R,
                           kind="ExternalInput").ap()
    one_v = nc.dram_tensor("one_v", [128, NK], F32R, kind="ExternalInput").ap()
    emb_a = nc.dram_tensor("emb_a", [3 * BC, S], F32, kind="ExternalInput").ap()
    out = nc.dram_tensor("out", [BC, S], F32, kind="ExternalOutput").ap()

    # SBUF (~173 KB/partition of 192)
    enc_sb = [nc.alloc_sbuf_tensor(f"enc{i}", [128, NK * CW], F32R).ap()
              for i in range(NSLOT)]
    m_sb = [[nc.alloc_sbuf_tensor(f"m{s_}_{j}", [128, H], F32R).ap()
             for j in range(NK)] for s_ in range(2)]
    ht_sb = nc.alloc_sbuf_tensor("ht", [128, NK * BC + 1], F32R).ap()
    aff_sb = nc.alloc_sbuf_tensor("aff", [128, NK * 3], F32R).ap()
    emb_sb = [nc.alloc_sbuf_tensor(f"emb{b}", [3, S], F32).ap()
              for b in range(BC)]
    p_sb = [nc.alloc_sbuf_tensor(f"p{i}", [128, NK * CH], F32R).ap()
            for i in range(2)]
    ones_sb = nc.alloc_sbuf_tensor("ones", [128, NK], F32R).ap()
    ha_sb = nc.alloc_sbuf_tensor("ha", [3, BC], F32).ap()
    e_sb = nc.alloc_sbuf_tensor("e", [128, S], F32).ap()
    e4_sb = nc.alloc_sbuf_tensor("e4", [BC, S], F32).ap()
    ex4_sb = nc.alloc_sbuf_tensor("ex4", [BC, S], F32).ap()
    o4_sb = nc.alloc_sbuf_tensor("o4", [BC, S], F32).ap()
    nmx_sb = nc.alloc_sbuf_tensor("nmx", [BC, 1], F32).ap()
    sm_sb = nc.alloc_sbuf_tensor("sm", [BC, 1], F32).ap()
    rs_sb = nc.alloc_sbuf_tensor("rs", [BC, 1], F32).ap()

    # PSUM: A region 4 banks + 2 e banks + 1 ha bank = 7 of 8
    ps_a = nc.alloc_psum_tensor("psA", [128, NK * CH], F32).ap()
    ps_e = [nc.alloc_psum_tensor(f"psE{i}", [128, CH], F32).ap()
            for i in range(2)]
    ps_ha = nc.alloc_psum_tensor("psHA", [128, CH], F32).ap()

    dma_set = nc.alloc_semaphore("dma_set")  # setup + M DMAs, serialized chain
    dma_ln = [nc.alloc_semaphore(f"dma_ln{k}") for k in range(NLANE)]
    dma_out = nc.alloc_semaphore("dma_out")
    dma_g = nc.alloc_semaphore("dma_g")      # e-row gather DMA
    pe_mm = nc.alloc_semaphore("pe_mm")      # +1 per kt MM-group (4/step)
    pe_red = nc.alloc_semaphore("pe_red")    # +1 per step reduce group
    pe_ha = nc.alloc_semaphore("pe_ha")      # +1 per batch ha-MM group
    dve_pm = nc.alloc_semaphore("dve_pm")    # +1 per P-mul pair (2/step)
    dve_aff = nc.alloc_semaphore("dve_aff")  # +1 per step (aff folded in P)
    dve_ms = nc.alloc_semaphore("dve_ms")    # +1 per DVE col-0 copy
    dve_sm = nc.alloc_semaphore("dve_sm")    # +1 reciprocal done (self-sync)
    dve_fin = nc.alloc_semaphore("dve_fin")  # +1 nmx ready, +1 final scale
    act_ha = nc.alloc_semaphore("act_ha")    # +1 per batch ha copied
    act_e = nc.alloc_semaphore("act_e")      # +1 per step e copied
    act_ex = nc.alloc_semaphore("act_ex")    # +1 exp done

    def m_thresh(b):
        return 16 * (SETUP_DMAS + 4 * (b + 1))

    with nc.Block() as blk:
        # --- SP: all DMAs (issue order fixed; per-lane completion chains) ---
        @blk.sync
        def _(sync):
            setup_srcs = [
                (ht_sb[:], h_t[:]),
                (aff_sb[:], aff_p[:]),
                *[(emb_sb[b][:], emb_a[3 * b:3 * b + 3, :])
                  for b in range(BC)],
                (ones_sb[:], one_v[:]),
                # seed col 0 of each block of each slot (c==0 chunks leave it
                # unwritten; fp32r MMs read a full even window; the value is
                # replaced in psum before use)
                *[(enc_sb[s_].rearrange("p (k w) -> p k w", k=NK)[:, :, 0:1],
                   one_v.rearrange("p (k o) -> p k o", o=1))
                  for s_ in range(NSLOT)],
            ]
            nset = 0
            with nc.allow_non_contiguous_dma(
                    reason="4x tiny one-time slot col-0 seeds (16 elems)"):
                for dst, src in setup_srcs:
                    sync.dma_start(dst, src).then_inc(dma_set, 16)
                    nset += 1
            def issue_m_set(b):
                nonlocal nset
                if b >= 2:  # WAR: batch b-2's slot free once its MMs done
                    sync.wait_ge(pe_mm, 32 * (b - 1))
                # all prior dma_set updates complete before this set issues,
                # so the per-batch full-sum threshold is order-unambiguous
                sync.wait_ge(dma_set, 16 * nset)
                for j in range(NK):
                    sync.dma_start(m_sb[b % 2][j][:],
                                   m_hb[b, j * 128:(j + 1) * 128, :]) \
                        .then_inc(dma_set, 16)
                    nset += 1
            # M sets for b=0,1 upfront; b+2's set is woven in after batch
            # b+1's chunk DMAs (its WAR wait needs batch b+1 fully issued)
            issue_m_set(0)
            issue_m_set(1)
            # chunk DMAs: ONE per step, 3D AP (p, kblock, s)
            for b in range(BC):
                for c in range(NCH):
                    # weave next-next batch's M set in mid-batch: SP is at most
                    # NSLOT steps ahead of PE here, so the WAR wait
                    # (pe_mm >= 32*b = batches < b done) is already satisfied
                    if b + 1 < BC and b >= 1 and c == NCH // 2:
                        issue_m_set(b + 1)
                    bc = b * NCH + c
                    if bc >= NSLOT:  # slot reuse: consumers of bc-4 done
                        sync.wait_ge(pe_mm, 4 * (bc - NSLOT) + 4)
                        sync.wait_ge(dve_pm, 2 * (bc - NSLOT) + 2)
                    if bc >= NLANE:  # lane chain => ordered completions
                        sync.wait_ge(dma_ln[bc % NLANE], 16 * (bc // NLANE))
                    # block kt col u holds s = c*CH - 1 + u; c==0: first real
                    # column lands at u=1 (col 0 pre-seeded)
                    s0 = c * CH - 1
                    u0, ncols = 0, CW
                    if c == 0:
                        s0, u0, ncols = 0, 1, CH
                    dst3 = enc_sb[slot(bc)].rearrange(
                        "p (k w) -> p k w", k=NK)[:, :, u0:u0 + ncols]
                    src3 = enc_t[b, :, s0:s0 + ncols].rearrange(
                        "(k p) s -> p k s", p=128)
                    sync.dma_start(dst3, src3).then_inc(dma_ln[bc % NLANE], 16)
            # gather e rows {0,32,64,96} -> contiguous [4, S] (DMA APs may
            # stride partitions; engine compute APs may not)
            sync.wait_ge(act_e, NBC)
            sync.dma_start(e4_sb[:], e_sb[0:128:32, :]).then_inc(dma_g, 16)
            sync.wait_ge(dve_fin, 2)
            sync.dma_start(out[:], o4_sb[:]).then_inc(dma_out, 16)
            sync.wait_ge(dma_out, 16)

        # --- PE ---
        @blk.tensor
        def _(tensor):
            def pe_reduce(j):
                # e_tmp[0, t] = sum_kt ones^T @ P_kt for step j
                tensor.wait_ge(dve_pm, 2 * j + 2)
                tensor.wait_ge(dve_aff, j + 1)
                if j >= 2:
                    tensor.wait_ge(act_e, j - 1)  # WAR on ps_e[j%2]
                pe_bank = ps_e[j % 2]
                for kt in range(NK):
                    mm_r = nc.tensor.matmul(
                        pe_bank[0:1, 0:CH], ones_sb[:, 0:1],
                        p_sb[j % 2][:, kt * CH:(kt + 1) * CH],
                        start=(kt == 0), stop=(kt == NK - 1))
                mm_r.then_inc(pe_red, 1)

            tensor.wait_ge(dma_set, 16 * SETUP_DMAS)  # setup inputs ready
            for b in range(BC):
                ms = m_sb[b % 2]
                tensor.wait_ge(dma_set, m_thresh(b))  # this batch's M_b ready
                # ha_b = affect^T @ h_b -> psum [3, 2] (fp32r needs even N)
                if b > 0:
                    tensor.wait_ge(act_ha, b)  # WAR on ps_ha
                for j in range(NK):
                    mm_ha = nc.tensor.matmul(
                        ps_ha[0:3, 0:2],
                        aff_sb[:, 3 * j:3 * j + 3],
                        ht_sb[:, NK * j + b:NK * j + b + 2],
                        start=(j == 0), stop=(j == NK - 1),
                    )
                mm_ha.then_inc(pe_ha, 1)
                for c in range(NCH):
                    bc = b * NCH + c
                    tensor.wait_ge(dma_ln[bc % NLANE],
                                   16 * (bc // NLANE + 1))  # chunk tile in
                    for kt in range(NK):
                        g = 4 * bc + kt
                        if g >= 4:  # WAR on psA bank kt: P-mul pair done
                            gp = g - 4
                            tensor.wait_ge(dve_pm,
                                           2 * (gp // 4) + (gp % 4) // 2 + 1)
                        for j in range(NK):
                            mm = nc.tensor.matmul(
                                ps_a[:, kt * CH:(kt + 1) * CH],
                                ms[j][:, kt * 128:(kt + 1) * 128],
                                enc_sb[slot(bc)][:, j * CW:j * CW + CH],
                                start=(j == 0), stop=(j == NK - 1),
                            )
                        mm.then_inc(pe_mm, 1)
                    # deferred reduce of the PREVIOUS step: its DVE pairs and
                    # aff finished during this step's MM groups -> no PE stall
                    if bc >= 1:
                        pe_reduce(bc - 1)
            pe_reduce(NBC - 1)

        # --- DVE ---
        @blk.vector
        def _(vector):
            n_ms = 0
            for b in range(BC):
                for c in range(NCH):
                    bc = b * NCH + c
                    if bc >= 2:
                        vector.wait_ge(pe_red, bc - 1)  # WAR on p[bc%2]
                    for half in range(2):  # P-mul banks (0,1) then (2,3)
                        vector.wait_ge(pe_mm, 4 * bc + 2 * half + 2)
                        if c == 0:
                            # psum col 0 of each bank := h_k (A'[-1] = h)
                            for kt in (2 * half, 2 * half + 1):
                                nc.vector.tensor_copy(
                                    ps_a[:, kt * CH:kt * CH + 1],
                                    ht_sb[:, NK * kt + b:NK * kt + b + 1]) \
                                    .then_inc(dve_ms, 1)
                                n_ms += 1
                            vector.wait_ge(dve_ms, n_ms)
                        pa3 = ps_a.rearrange("p (k s) -> p k s", k=NK)[
                            :, 2 * half:2 * half + 2, :]
                        en3 = enc_sb[slot(bc)].rearrange(
                            "p (k w) -> p k w", k=NK)[
                            :, 2 * half:2 * half + 2, 1:CW]
                        po3 = p_sb[bc % 2].rearrange(
                            "p (k s) -> p k s", k=NK)[
                            :, 2 * half:2 * half + 2, :]
                        nc.vector.tensor_mul(po3, pa3, en3) \
                            .then_inc(dve_pm, 1)
                    # affect term into P rows 0..2 (after pair 0 completes)
                    vector.wait_ge(act_ha, b + 1)   # ha_sb[., b] ready
                    vector.wait_ge(dve_pm, 2 * bc + 1)
                    nc.vector.scalar_tensor_tensor(
                        p_sb[bc % 2][0:3, 0:CH],
                        emb_sb[b][0:3, c * CH:(c + 1) * CH],
                        ha_sb[0:3, b:b + 1],
                        p_sb[bc % 2][0:3, 0:CH],
                        mybir.AluOpType.mult, mybir.AluOpType.add,
                    ).then_inc(dve_aff, 1)
            # softmax (batched on contiguous [4, S])
            vector.wait_ge(dma_g, 16)
            nc.vector.tensor_reduce(nmx_sb[:], e4_sb[:], mybir.AxisListType.X,
                                    mybir.AluOpType.max, negate=True) \
                .then_inc(dve_fin, 1)   # "nmx ready" (ACT waits 1)
            vector.wait_ge(act_ex, 1)
            nc.vector.reciprocal(rs_sb[:], sm_sb[:]).then_inc(dve_sm, 1)
            vector.wait_ge(dve_sm, 1)
            nc.vector.tensor_scalar_mul(o4_sb[:], ex4_sb[:], rs_sb[0:BC, 0:1]) \
                .then_inc(dve_fin, 1)   # dve_fin==2 -> SP may DMA out

        # --- ACT: PSUM->SBUF copies, exp ---
        @blk.scalar
        def _(scalar):
            for b in range(BC):
                scalar.wait_ge(pe_ha, b + 1)
                nc.scalar.copy(ha_sb[0:3, b:b + 1], ps_ha[0:3, 0:1]) \
                    .then_inc(act_ha, 1)
                for c in range(NCH):
                    bc = b * NCH + c
                    scalar.wait_ge(pe_red, bc + 1)
                    nc.scalar.copy(e_sb[32 * b:32 * b + 1, c * CH:(c + 1) * CH],
                                   ps_e[bc % 2][0:1, 0:CH]).then_inc(act_e, 1)
            scalar.wait_ge(dve_fin, 1)  # nmx ready
            nc.scalar.activation(ex4_sb[:], e4_sb[:],
                                 mybir.ActivationFunctionType.Exp,
                                 bias=nmx_sb[0:BC, 0:1],
                                 accum_out=sm_sb[0:BC, 0:1]) \
                .then_inc(act_ex, 1)

    # no end-of-program sem clears: each PJRT execution starts with fresh
    # semaphore state (verified: 3 back-to-back executions of one loaded NEFF
    # each gave correct, input-scaled results).
    return nc


def _shard_host(hidden, encoder_outputs, embedding, bigram_matrix, affect_matrix):
    """Build per-core input maps. Only layout/scaling prep happens here."""
    h = np.asarray(hidden, dtype=np.float32)[0]              # [B, H]
    enc = np.asarray(encoder_outputs, dtype=np.float32)      # [S, B, H]
    emb = np.asarray(embedding, dtype=np.float32)            # [S, B, 3]
    m = np.ascontiguousarray(np.asarray(bigram_matrix, dtype=np.float32))
    aff = np.asarray(affect_matrix, dtype=np.float32)        # [H, 3]

    enc_bhs = np.ascontiguousarray(enc.transpose(1, 2, 0))   # [B, H, S]
    emb_bks = np.ascontiguousarray(emb.transpose(1, 2, 0))   # [B, 3, S]
    aff_pk = np.ascontiguousarray(
        aff.reshape(NK, 128, 3).transpose(1, 0, 2).reshape(128, NK * 3))
    # h folded into M per batch: m_hb[b, j, k] = M[j, k] * h[b, k]
    m_hb_all = np.ascontiguousarray(m[None, :, :] * h[:, None, :])  # [B,H,H]

    in_maps = []
    for co in range(NCORES):
        b0 = co * BC
        h_sl = h[b0:b0 + BC]                                  # [BC, H]
        ht_pk = np.concatenate([
            h_sl.reshape(BC, NK, 128).transpose(2, 1, 0).reshape(128, NK * BC),
            np.zeros((128, 1), dtype=np.float32)], axis=1)
        in_maps.append({
            "enc_t": enc_bhs[b0:b0 + BC],                     # [BC, H, S]
            "m_hb": m_hb_all[b0:b0 + BC],
            "h_t": np.ascontiguousarray(ht_pk),
            "aff_p": aff_pk,
            "one_v": np.ones((128, NK), dtype=np.float32),
            "emb_a": emb_bks[b0:b0 + BC].reshape(3 * BC, S),
        })
    return in_maps


def kernel(hidden, encoder_outputs, embedding, bigram_matrix, affect_matrix,
           _want_results=False, _spmd_kwargs=None):
    nc = _build()
    in_maps = _shard_host(hidden, encoder_outputs, embedding,
                          bigram_matrix, affect_matrix)
    res = run_bass_kernel_spmd(nc, in_maps, core_ids=list(range(NCORES)),
                               **(_spmd_kwargs or {}))
    outp = np.empty((B, 1, S), dtype=np.float32)
    for co in range(NCORES):
        outp[co * BC:(co + 1) * BC, 0, :] = res.results[co]["out"]
    if _want_results:
        return outp, res
    return outp



# revision 28
# speedup vs baseline: 1.0585x; 1.0585x over previous
"""BigramAttn Trainium2 kernel (8-core SPMD, raw Bass) — fp16 pipeline.

Reference computation (per batch b):
  e[0]   = sum_k enc[0,k] * h[k]
  e[s]   = sum_k (enc[s-1,:] @ M)[k] * h[k] * enc[s,k]          (s >= 1)
  e[s]  += sum_{k<3} (h @ affect)[k] * emb[s,k]
  out    = softmax(e)                                            # over s

Sharding: data-parallel over batch B=32 across 8 cores (4 batches/core).

fp16 data path (measured end-to-end rel err ~4e-3 vs the 2e-2 gate):
enc/M/emb/ha ship as fp16 (halves HBM traffic vs fp32; total ~17.5MB/core),
all matmuls are fp16 in / fp32 psum accumulate. h stays fp32 and is folded
on device (scalar_tensor_tensor per-partition scalar), so M is a single
shared 512KB load instead of per-batch M*diag(h) copies.

Per chunk-step (b, c) over a 513-wide enc window (1-col halo, host-packed
contiguous so each step is ONE 525KB DMA):
  PE:   A_kt[k,t] = sum_j M^T enc_prev      (16 fp16 MMs -> ps_a, 4 banks)
  ACT:  Y_01 = fp16(h_01 * A_01)            (copy+scale psum->sbuf)
  DVE:  P01  = Y01 * enc_01                 (fp16 TT, 2x mode)
  DVE:  P2,P3 = (A_kt*h_kt)*enc_kt          (stt, psum 1x; GPSIMD can't
                                             touch PSUM on trn2)
  POOL: Q = (P0+P1) + (P2+P3)               (fp16 add tree, SBUF only)
  PE:   e = ones^T Q + ha^T emb_c           (2 accumulating MMs -> ps_e)
  DVE:  nm_c = -max(e)                      (per-chunk max)
  ACT:  ex_c = exp(e + nm_c), sm_c = sum    (flash-style, overlapped)
Tail per batch: m_b = max_c mx_c; ed_c = exp(mx_c-m_b); Z_b = sum ed_c*sm_c;
alpha_c = ed_c/Z_b; out = ex_c * alpha_c (rescale split DVE/ACT/POOL).

This walrus build accepts exactly ONE semaphore wait per instruction, so the
kernel is raw Bass: per-engine programs, counting semaphores, standalone
waits. DMA completions may reorder across transfers, so chunk DMAs chain on
per-lane semaphores.
"""

import functools

import numpy as np

import concourse.bass as bass
from concourse import mybir
from concourse.bass_utils import run_bass_kernel_spmd

S, B, H = 4096, 32, 512
NCORES = 8
BC = B // NCORES          # batches per core = 4
NK = H // 128             # h-chunks = 4
CH = 512                  # s-chunk width
CW = CH + 1               # packed chunk block width (1-col halo)
NCH = S // CH             # s-chunks per batch = 8
NBC = BC * NCH            # chunk-steps per core = 32
NSLOT = 4                 # enc chunk tiles in flight
NLANE = 4                 # DMA completion-ordering lanes
NEB = 3                   # ps_e rotation depth
SETUP_DMAS = 9            # m, ht, ha, ones16, ones32, 4x emb

F32 = mybir.dt.float32
F16 = mybir.dt.float16


@functools.lru_cache(maxsize=1)
def _build():
    nc = bass.Bass("TRN2", target_bir_lowering=False, debug=False)

    enc_c = nc.dram_tensor("enc_c", [NBC, 128, NK * CW], F16,
                           kind="ExternalInput").ap()
    m_p = nc.dram_tensor("m_p", [128, NK * H], F16, kind="ExternalInput").ap()
    h_t = nc.dram_tensor("h_t", [128, BC * NK], F32, kind="ExternalInput").ap()
    ha_p = nc.dram_tensor("ha_p", [3, BC], F16, kind="ExternalInput").ap()
    one_h = nc.dram_tensor("one_h", [128, 1], F16, kind="ExternalInput").ap()
    one_f = nc.dram_tensor("one_f", [128, 1], F32, kind="ExternalInput").ap()
    emb_a = nc.dram_tensor("emb_a", [3 * BC, S], F16, kind="ExternalInput").ap()
    out = nc.dram_tensor("out", [BC, S], F32, kind="ExternalOutput").ap()

    # SBUF
    enc_sb = [nc.alloc_sbuf_tensor(f"enc{i}", [128, NK * CW], F16).ap()
              for i in range(NSLOT)]
    m_sb = nc.alloc_sbuf_tensor("m", [128, NK * H], F16).ap()
    ht_sb = nc.alloc_sbuf_tensor("ht", [128, BC * NK], F32).ap()
    ha_sb = nc.alloc_sbuf_tensor("ha", [3, BC], F16).ap()
    oh_sb = nc.alloc_sbuf_tensor("oh", [128, 1], F16).ap()
    of_sb = nc.alloc_sbuf_tensor("of", [128, 1], F32).ap()
    emb_sb = [nc.alloc_sbuf_tensor(f"emb{b}", [3, S], F16).ap()
              for b in range(BC)]
    y_sb = [nc.alloc_sbuf_tensor(f"y{i}", [128, 2 * CH], F16).ap()
            for i in range(2)]
    p_sb = [nc.alloc_sbuf_tensor(f"p{i}", [128, NK * CH], F16).ap()
            for i in range(2)]
    q_sb = [nc.alloc_sbuf_tensor(f"q{i}", [128, 3 * CH], F16).ap()
            for i in range(2)]
    ex_w = nc.alloc_sbuf_tensor("ex_w", [128, S], F32).ap()
    aw = nc.alloc_sbuf_tensor("aw", [128, NCH], F32).ap()
    ex4 = nc.alloc_sbuf_tensor("ex4", [BC, S], F32).ap()
    o4 = nc.alloc_sbuf_tensor("o4", [BC, S], F32).ap()
    nm0 = nc.alloc_sbuf_tensor("nm0", [1, NBC], F32).ap()   # -chunk max
    sm0 = nc.alloc_sbuf_tensor("sm0", [1, NBC], F32).ap()   # chunk expsum
    mnb = nc.alloc_sbuf_tensor("mnb", [1, BC], F32).ap()    # min_c nm = -m_b
    ed0 = nc.alloc_sbuf_tensor("ed0", [1, NBC], F32).ap()   # exp(mx_c-m_b)
    w0 = nc.alloc_sbuf_tensor("w0", [1, NBC], F32).ap()
    zb = nc.alloc_sbuf_tensor("zb", [1, BC], F32).ap()
    rz = nc.alloc_sbuf_tensor("rz", [1, BC], F32).ap()
    al4 = nc.alloc_sbuf_tensor("al4", [BC, NCH], F32).ap()

    # PSUM: 4 banks A + 3 banks e = 7 of 8
    ps_a = nc.alloc_psum_tensor("psA", [128, NK * CH], F32).ap()
    ps_e = [nc.alloc_psum_tensor(f"psE{i}", [128, CH], F32).ap()
            for i in range(NEB)]

    dma_set = nc.alloc_semaphore("dma_set")
    dma_ln = [nc.alloc_semaphore(f"dma_ln{k}") for k in range(NLANE)]
    dma_g = nc.alloc_semaphore("dma_g")
    dma_out = nc.alloc_semaphore("dma_out")
    pe_mm = nc.alloc_semaphore("pe_mm")      # +1 per kt MM-group (4/step)
    pe_red = nc.alloc_semaphore("pe_red")    # +1 per step e-reduce
    act_y = nc.alloc_semaphore("act_y")      # +1 per Y half (2/step)
    act_exp = nc.alloc_semaphore("act_exp")  # +1 per chunk exp
    act_ed = nc.alloc_semaphore("act_ed")    # +1 per batch ed
    act_fin = nc.alloc_semaphore("act_fin")
    dve_p = nc.alloc_semaphore("dve_p")      # +1 per step P01 mul
    dve_s = nc.alloc_semaphore("dve_s")      # +1 per stt (2/step)
    dve_sd = nc.alloc_semaphore("dve_sd")    # DVE col-0 seed self-sync
    dve_mx = nc.alloc_semaphore("dve_mx")    # +1 per chunk max
    dve_tl = nc.alloc_semaphore("dve_tl")    # +1 per batch mnb
    dve_tc = nc.alloc_semaphore("dve_tc")    # tail chain self-sync counter
    dve_al = nc.alloc_semaphore("dve_al")    # +1 alphas ready
    dve_fin = nc.alloc_semaphore("dve_fin")
    act_sd = nc.alloc_semaphore("act_sd")    # ACT col-0 seed self-sync
    pool_t2 = nc.alloc_semaphore("pool_t2")  # +2 per step (Q1, Q2)
    pool_q = nc.alloc_semaphore("pool_q")    # +1 per step Q done
    pool_fin = nc.alloc_semaphore("pool_fin")

    EXP = mybir.ActivationFunctionType.Exp

    with nc.Block() as blk:
        # --- SP: all DMAs ---
        @blk.sync
        def _(sync):
            setup = [
                (m_sb[:], m_p[:]),
                (ht_sb[:], h_t[:]),
                (ha_sb[:], ha_p[:]),
                (oh_sb[:], one_h[:]),
                (of_sb[:], one_f[:]),
                *[(emb_sb[b][:], emb_a[3 * b:3 * b + 3, :])
                  for b in range(BC)],
            ]
            for dst, src in setup:
                sync.dma_start(dst, src).then_inc(dma_set, 16)
            for bc in range(NBC):
                if bc >= NSLOT:
                    p = bc - NSLOT
                    sync.wait_ge(pe_mm, 4 * p + 4)
                    sync.wait_ge(dve_p, p + 1)
                    sync.wait_ge(dve_s, 2 * p + 2)
                if bc >= NLANE:
                    sync.wait_ge(dma_ln[bc % NLANE], 16 * (bc // NLANE))
                sync.dma_start(enc_sb[bc % NSLOT][:], enc_c[bc]) \
                    .then_inc(dma_ln[bc % NLANE], 16)
            # gather strided batch rows to contiguous [4, .] for the rescale
            sync.wait_ge(act_exp, NBC)
            sync.dma_start(ex4[:], ex_w[0:128:32, :]).then_inc(dma_g, 16)
            sync.wait_ge(dve_al, 1)
            sync.dma_start(al4[:], aw[0:128:32, :]).then_inc(dma_g, 16)
            sync.wait_ge(dve_fin, 1)
            sync.wait_ge(act_fin, 1)
            sync.wait_ge(pool_fin, 1)
            sync.dma_start(out[:], o4[:]).then_inc(dma_out, 16)
            sync.wait_ge(dma_out, 16)

        # --- PE ---
        @blk.tensor
        def _(tensor):
            def reduce(j):
                b, c, r, par = j // NCH, j % NCH, j % NEB, j % 2
                tensor.wait_ge(pool_q, j + 1)
                if j >= NEB:
                    tensor.wait_ge(act_exp, j - NEB + 1)  # WAR ps_e[r]
                nc.tensor.matmul(
                    ps_e[r][0:1, 0:CH], oh_sb[:, 0:1],
                    q_sb[par][:, 2 * CH:3 * CH],
                    start=True, stop=False)
                nc.tensor.matmul(
                    ps_e[r][0:1, 0:CH], ha_sb[0:3, b:b + 1],
                    emb_sb[b][0:3, c * CH:(c + 1) * CH],
                    start=False, stop=True).then_inc(pe_red, 1)

            tensor.wait_ge(dma_set, 16 * SETUP_DMAS)
            for bc in range(NBC):
                slot = bc % NSLOT
                tensor.wait_ge(dma_ln[bc % NLANE], 16 * (bc // NLANE + 1))
                for kt in range(NK):
                    # per-bank WAR: wait only for the drain of THIS bank from
                    # the previous step, so late stt's don't stall early MMs
                    if bc >= 1:
                        if kt == 0:
                            tensor.wait_ge(act_y, 2 * bc)       # Y of bc-1
                        elif kt == 2:
                            tensor.wait_ge(dve_s, 2 * bc - 1)   # stt P2
                        elif kt == 3:
                            tensor.wait_ge(dve_s, 2 * bc)       # stt P3
                    for j in range(NK):
                        mm = nc.tensor.matmul(
                            ps_a[:, kt * CH:(kt + 1) * CH],
                            m_sb[:, j * H + kt * 128:j * H + (kt + 1) * 128],
                            enc_sb[slot][:, j * CW:j * CW + CH],
                            start=(j == 0), stop=(j == NK - 1))
                    mm.then_inc(pe_mm, 1)
                if bc >= 1:
                    reduce(bc - 1)
            reduce(NBC - 1)

        # --- ACT: Y copies (h-fold, psum->sbuf fp16), chunk exp, batch ed ---
        @blk.scalar
        def _(scalar):
            def exp_op(j):
                b, c, r = j // NCH, j % NCH, j % NEB
                scalar.wait_ge(pe_red, j + 1)
                scalar.wait_ge(dve_mx, j + 1)
                nc.scalar.activation(
                    ex_w[32 * b:32 * b + 1, c * CH:(c + 1) * CH],
                    ps_e[r][0:1, 0:CH],
                    EXP, bias=nm0[0:1, j:j + 1],
                    accum_out=sm0[0:1, j:j + 1]).then_inc(act_exp, 1)

            def ed_op(b):
                scalar.wait_ge(dve_tl, b + 1)
                nc.scalar.activation(
                    ed0[0:1, NCH * b:NCH * (b + 1)],
                    nm0[0:1, NCH * b:NCH * (b + 1)],
                    EXP, bias=mnb[0:1, b:b + 1], scale=-1.0) \
                    .then_inc(act_ed, 1)

            scalar.wait_ge(dma_set, 16 * SETUP_DMAS)
            n_sd = 0
            for bc in range(NBC):
                b, c, par = bc // NCH, bc % NCH, bc % 2
                scalar.wait_ge(pe_mm, 4 * bc + 1)
                if bc >= 2:
                    scalar.wait_ge(dve_p, bc - 1)    # WAR y_sb[par]
                if c == 0:
                    nc.scalar.copy(ps_a[:, 0:1], of_sb[:]) \
                        .then_inc(act_sd, 1)
                    n_sd += 1
                    scalar.wait_ge(act_sd, n_sd)
                nc.scalar.mul(y_sb[par][:, 0:CH], ps_a[:, 0:CH],
                              ht_sb[:, NK * b:NK * b + 1]).then_inc(act_y, 1)
                scalar.wait_ge(pe_mm, 4 * bc + 2)
                if c == 0:
                    nc.scalar.copy(ps_a[:, CH:CH + 1], of_sb[:]) \
                        .then_inc(act_sd, 1)
                    n_sd += 1
                    scalar.wait_ge(act_sd, n_sd)
                nc.scalar.mul(y_sb[par][:, CH:2 * CH], ps_a[:, CH:2 * CH],
                              ht_sb[:, NK * b + 1:NK * b + 2]) \
                    .then_inc(act_y, 1)
                # lag-2 exp: its dve_mx dependency lands late in step bc-1,
                # so a lag-1 exp would stall ACT and delay the next Y pair
                if bc >= 2:
                    exp_op(bc - 2)
                if bc % NCH == 1 and bc > NCH:
                    ed_op(bc // NCH - 1)
            exp_op(NBC - 2)
            exp_op(NBC - 1)
            ed_op(BC - 1)
            # rescale chunks 4,5
            scalar.wait_ge(dma_g, 32)
            nc.scalar.mul(o4[:, 4 * CH:5 * CH], ex4[:, 4 * CH:5 * CH],
                          al4[0:BC, 4:5])
            nc.scalar.mul(o4[:, 5 * CH:6 * CH], ex4[:, 5 * CH:6 * CH],
                          al4[0:BC, 5:6]).then_inc(act_fin, 1)

        # --- DVE: P01 mul, Q tree, chunk max, tail combine, rescale ---
        @blk.vector
        def _(vector):
            def mx_op(j):
                r = j % NEB
                vector.wait_ge(pe_red, j + 1)
                nc.vector.tensor_reduce(
                    nm0[0:1, j:j + 1], ps_e[r][0:1, 0:CH],
                    mybir.AxisListType.X, mybir.AluOpType.max,
                    negate=True).then_inc(dve_mx, 1)

            n_tc = 0

            def mnb_op(b):
                vector.wait_ge(dve_mx, NCH * (b + 1))  # own nm0 writes acked
                nc.vector.tensor_reduce(
                    mnb[0:1, b:b + 1], nm0[0:1, NCH * b:NCH * (b + 1)],
                    mybir.AxisListType.X, mybir.AluOpType.min) \
                    .then_inc(dve_tl, 1)

            def wz_op(b):
                nonlocal n_tc
                vector.wait_ge(act_ed, b + 1)
                vector.wait_ge(act_exp, NCH * (b + 1))
                nc.vector.tensor_mul(w0[0:1, NCH * b:NCH * (b + 1)],
                                     ed0[0:1, NCH * b:NCH * (b + 1)],
                                     sm0[0:1, NCH * b:NCH * (b + 1)]) \
                    .then_inc(dve_tc, 1)
                n_tc += 1
                vector.wait_ge(dve_tc, n_tc)
                nc.vector.tensor_reduce(
                    zb[0:1, b:b + 1], w0[0:1, NCH * b:NCH * (b + 1)],
                    mybir.AxisListType.X, mybir.AluOpType.add) \
                    .then_inc(dve_tc, 1)
                n_tc += 1

            n_sd = 0
            for bc in range(NBC):
                b, c, par, slot = bc // NCH, bc % NCH, bc % 2, bc % NSLOT
                # P01 = Y01 * E01
                vector.wait_ge(act_y, 2 * bc + 2)
                if bc >= 2:
                    vector.wait_ge(pool_q, bc - 1)   # WAR p_sb[par]
                nc.vector.tensor_mul(
                    p_sb[par].rearrange("p (k s) -> p k s", k=NK)[:, 0:2, :],
                    y_sb[par].rearrange("p (k s) -> p k s", k=2)[:, :, :],
                    enc_sb[slot].rearrange("p (k w) -> p k w", k=NK)
                    [:, 0:2, 1:CW]).then_inc(dve_p, 1)
                # P2, P3 stt folds (psum)
                for kt in (2, 3):
                    vector.wait_ge(pe_mm, 4 * bc + kt + 1)
                    if c == 0:
                        nc.vector.tensor_copy(
                            ps_a[:, kt * CH:kt * CH + 1], of_sb[:]) \
                            .then_inc(dve_sd, 1)
                        n_sd += 1
                        vector.wait_ge(dve_sd, n_sd)
                    nc.vector.scalar_tensor_tensor(
                        p_sb[par][:, kt * CH:(kt + 1) * CH],
                        ps_a[:, kt * CH:(kt + 1) * CH],
                        ht_sb[:, NK * b + kt:NK * b + kt + 1],
                        enc_sb[slot][:, kt * CW + 1:kt * CW + CW],
                        mybir.AluOpType.mult, mybir.AluOpType.mult) \
                        .then_inc(dve_s, 1)
                if bc >= 1:
                    mx_op(bc - 1)
                if bc % NCH == 0 and bc >= NCH:
                    mnb_op(bc // NCH - 1)
                if bc % NCH == 1 and bc >= NCH:
                    wz_op(bc // NCH - 1)
            mx_op(NBC - 1)
            mnb_op(BC - 1)
            wz_op(BC - 1)
            vector.wait_ge(dve_tc, n_tc)             # zb writes acked
            nc.vector.reciprocal(rz[0:1, 0:BC], zb[0:1, 0:BC]) \
                .then_inc(dve_tc, 1)
            n_tc += 1
            vector.wait_ge(dve_tc, n_tc)             # rz write acked
            for b in range(BC):
                op = nc.vector.tensor_scalar_mul(
                    aw[32 * b:32 * b + 1, 0:NCH],
                    ed0[0:1, NCH * b:NCH * (b + 1)], rz[0:1, b:b + 1])
            op.then_inc(dve_al, 1)
            # rescale chunks 0-3
            vector.wait_ge(dma_g, 32)
            for cc in range(4):
                op = nc.vector.tensor_scalar_mul(
                    o4[:, cc * CH:(cc + 1) * CH],
                    ex4[:, cc * CH:(cc + 1) * CH], al4[0:BC, cc:cc + 1])
            op.then_inc(dve_fin, 1)

        # --- POOL (gpsimd): P2,P3 stt folds, rescale chunks 6,7 ---
        @blk.gpsimd
        def _(gpsimd):
            for bc in range(NBC):
                par = bc % 2
                if bc >= 2:
                    gpsimd.wait_ge(pe_red, bc - 1)   # WAR q_sb[par] (Q slice)
                gpsimd.wait_ge(dve_p, bc + 1)        # P01 landed
                nc.gpsimd.tensor_add(q_sb[par][:, 0:CH],
                                     p_sb[par][:, 0:CH],
                                     p_sb[par][:, CH:2 * CH]) \
                    .then_inc(pool_t2, 1)
                gpsimd.wait_ge(dve_s, 2 * bc + 2)    # P2, P3 landed
                nc.gpsimd.tensor_add(q_sb[par][:, CH:2 * CH],
                                     p_sb[par][:, 2 * CH:3 * CH],
                                     p_sb[par][:, 3 * CH:4 * CH]) \
                    .then_inc(pool_t2, 1)
                gpsimd.wait_ge(pool_t2, 2 * bc + 2)  # own writes acked
                nc.gpsimd.tensor_add(q_sb[par][:, 2 * CH:3 * CH],
                                     q_sb[par][:, 0:CH],
                                     q_sb[par][:, CH:2 * CH]) \
                    .then_inc(pool_q, 1)
            # rescale chunks 6,7
            gpsimd.wait_ge(dma_g, 32)
            for cc in (6, 7):
                op = nc.gpsimd.tensor_scalar_mul(
                    o4[:, cc * CH:(cc + 1) * CH],
                    ex4[:, cc * CH:(cc + 1) * CH], al4[0:BC, cc:cc + 1])
            op.then_inc(pool_fin, 1)

    return nc


def _shard_host(hidden, encoder_outputs, embedding, bigram_matrix, affect_matrix):
    """Per-core input maps. Layout/cast prep only (plus tiny h@affect)."""
    h = np.asarray(hidden, dtype=np.float32)[0]              # [B, H]
    enc = np.asarray(encoder_outputs, dtype=np.float32)      # [S, B, H]
    emb = np.asarray(embedding, dtype=np.float32)            # [S, B, 3]
    m = np.asarray(bigram_matrix, dtype=np.float32)
    aff = np.asarray(affect_matrix, dtype=np.float32)        # [H, 3]

    # padded fp16 enc: row 0 is the s=-1 halo for c==0 (value irrelevant;
    # psum col 0 is re-seeded on device)
    encp = np.zeros((S + 1, B, H), dtype=np.float16)
    encp[1:] = enc.astype(np.float16)

    m16 = m.astype(np.float16)
    m_p = np.ascontiguousarray(
        m16.reshape(NK, 128, H).transpose(1, 0, 2).reshape(128, NK * H))
    ha = (h @ aff).T.astype(np.float16)                      # [3, B]
    emb16 = np.ascontiguousarray(
        emb.transpose(1, 2, 0).astype(np.float16))           # [B, 3, S]
    one_h = np.ones((128, 1), dtype=np.float16)
    one_f = np.ones((128, 1), dtype=np.float32)

    in_maps = []
    for co in range(NCORES):
        b0 = co * BC
        # enc_c[b*NCH+c, p, k*CW+w] = encp[c*CH+w, b0+b, k*128+p]
        blocks = []
        for b in range(b0, b0 + BC):
            v = np.ascontiguousarray(encp[:, b, :])          # [S+1, H]
            w = np.lib.stride_tricks.as_strided(
                v, shape=(NCH, CW, H),
                strides=(CH * v.strides[0], v.strides[0], v.strides[1]))
            t = w.transpose(0, 2, 1).reshape(NCH, NK, 128, CW)
            blocks.append(t.transpose(0, 2, 1, 3).reshape(NCH, 128, NK * CW))
        enc_cc = np.ascontiguousarray(np.concatenate(blocks, axis=0))
        h_sl = h[b0:b0 + BC]                                 # [BC, H]
        ht = np.ascontiguousarray(
            h_sl.reshape(BC, NK, 128).transpose(2, 0, 1).reshape(128, BC * NK))
        in_maps.append({
            "enc_c": enc_cc,
            "m_p": m_p,
            "h_t": ht,
            "ha_p": np.ascontiguousarray(ha[:, b0:b0 + BC]),
            "one_h": one_h,
            "one_f": one_f,
            "emb_a": emb16[b0:b0 + BC].reshape(3 * BC, S),
        })
    return in_maps


def kernel(hidden, encoder_outputs, embedding, bigram_matrix, affect_matrix,
           _want_results=False, _spmd_kwargs=None):
    nc = _build()
    in_maps = _shard_host(hidden, encoder_outputs, embedding,
                          bigram_matrix, affect_matrix)
    res = run_bass_kernel_spmd(nc, in_maps, core_ids=list(range(NCORES)),
                               **(_spmd_kwargs or {}))
    outp = np.empty((B, 1, S), dtype=np.float32)
    for co in range(NCORES):
        outp[co * BC:(co + 1) * BC, 0, :] = res.results[co]["out"]
    if _want_results:
        return outp, res
    return outp


# revision 37
# speedup vs baseline: 1.2227x; 1.1551x over previous
"""BigramAttn Trainium2 kernel (8-core SPMD, raw Bass) — fp16 pipeline.

Reference computation (per batch b):
  e[0]   = sum_k enc[0,k] * h[k]
  e[s]   = sum_k (enc[s-1,:] @ M)[k] * h[k] * enc[s,k]          (s >= 1)
  e[s]  += sum_{k<3} (h @ affect)[k] * emb[s,k]
  out    = softmax(e)                                            # over s

Sharding: data-parallel over batch B=32 across 8 cores (4 batches/core).

fp16 data path (measured end-to-end rel err ~4e-3 vs the 2e-2 gate):
enc/M/emb/ha ship as fp16 (halves HBM traffic vs fp32; total ~17.5MB/core),
all matmuls are fp16 in / fp32 psum accumulate. h stays fp32 and is folded
on device (scalar_tensor_tensor per-partition scalar), so M is a single
shared 512KB load instead of per-batch M*diag(h) copies.

Per chunk-step (b, c) over a 513-wide enc window (1-col halo, host-packed
contiguous so each step is ONE 525KB DMA):
  PE:   A_kt[k,t] = sum_j M^T enc_prev      (16 fp16 MMs -> ps_a, 4 banks)
  ACT:  Y_01 = fp16(h_01 * A_01)            (copy+scale psum->sbuf)
  DVE:  P01  = Y01 * enc_01                 (fp16 TT, 2x mode)
  DVE:  P2,P3 = (A_kt*h_kt)*enc_kt          (stt, psum 1x; GPSIMD can't
                                             touch PSUM on trn2)
  POOL: Q = (P0+P1) + (P2+P3)               (fp16 add tree, SBUF only)
  PE:   e = ones^T Q + ha^T emb_c           (2 accumulating MMs -> ps_e)
  DVE:  nm_c = -max(e)                      (per-chunk max)
  ACT:  ex_c = exp(e + nm_c), sm_c = sum    (flash-style, overlapped)
Tail per batch: m_b = max_c mx_c; ed_c = exp(mx_c-m_b); Z_b = sum ed_c*sm_c;
alpha_c = ed_c/Z_b; out = ex_c * alpha_c (rescale split DVE/ACT/POOL).

This walrus build accepts exactly ONE semaphore wait per instruction, so the
kernel is raw Bass: per-engine programs, counting semaphores, standalone
waits. DMA completions may reorder across transfers, so chunk DMAs chain on
per-lane semaphores.
"""

import functools

import numpy as np

import concourse.bass as bass
from concourse import mybir
from concourse.bass_utils import run_bass_kernel_spmd

S, B, H = 4096, 32, 512
NCORES = 8
BC = B // NCORES          # batches per core = 4
NK = H // 128             # h-chunks = 4
CH = 512                  # s-chunk width
CW = CH + 1               # packed chunk block width (1-col halo)
NCH = S // CH             # s-chunks per batch = 8
NBC = BC * NCH            # chunk-steps per core = 32
NSLOT = 4                 # enc chunk tiles in flight
NLANE = 4                 # DMA completion-ordering lanes
NEB = 3                   # ps_e rotation depth
SETUP_DMAS = 9            # m, ht, ha, ones16, ones32, 4x emb

F32 = mybir.dt.float32
F16 = mybir.dt.float16


@functools.lru_cache(maxsize=1)
def _build():
    nc = bass.Bass("TRN2", target_bir_lowering=False, debug=False)

    enc_c = nc.dram_tensor("enc_c", [NBC, 128, NK * CW], F16,
                           kind="ExternalInput").ap()
    m_p = nc.dram_tensor("m_p", [128, NK * H], F16, kind="ExternalInput").ap()
    h_t = nc.dram_tensor("h_t", [128, BC * NK], F32, kind="ExternalInput").ap()
    ha_p = nc.dram_tensor("ha_p", [3, BC], F16, kind="ExternalInput").ap()
    one_h = nc.dram_tensor("one_h", [128, 1], F16, kind="ExternalInput").ap()
    one_f = nc.dram_tensor("one_f", [128, 1], F32, kind="ExternalInput").ap()
    emb_a = nc.dram_tensor("emb_a", [3 * BC, S], F16, kind="ExternalInput").ap()
    out = nc.dram_tensor("out", [BC, S], F32, kind="ExternalOutput").ap()

    # SBUF
    enc_sb = [nc.alloc_sbuf_tensor(f"enc{i}", [128, NK * CW], F16).ap()
              for i in range(NSLOT)]
    m_sb = nc.alloc_sbuf_tensor("m", [128, NK * H], F16).ap()
    ht_sb = nc.alloc_sbuf_tensor("ht", [128, BC * NK], F32).ap()
    ha_sb = nc.alloc_sbuf_tensor("ha", [3, BC], F16).ap()
    oh_sb = nc.alloc_sbuf_tensor("oh", [128, 1], F16).ap()
    of_sb = nc.alloc_sbuf_tensor("of", [128, 1], F32).ap()
    emb_sb = [nc.alloc_sbuf_tensor(f"emb{b}", [3, S], F16).ap()
              for b in range(BC)]
    y_sb = [nc.alloc_sbuf_tensor(f"y{i}", [128, 2 * CH], F16).ap()
            for i in range(2)]
    p_sb = [nc.alloc_sbuf_tensor(f"p{i}", [128, NK * CH], F16).ap()
            for i in range(2)]
    q_sb = [nc.alloc_sbuf_tensor(f"q{i}", [128, 3 * CH], F16).ap()
            for i in range(3)]
    ex_w = nc.alloc_sbuf_tensor("ex_w", [128, S], F32).ap()
    aw = nc.alloc_sbuf_tensor("aw", [128, NCH], F32).ap()
    ex4 = nc.alloc_sbuf_tensor("ex4", [BC, S], F32).ap()
    o4 = nc.alloc_sbuf_tensor("o4", [BC, S], F32).ap()
    nm0 = nc.alloc_sbuf_tensor("nm0", [1, NBC], F32).ap()   # -chunk max
    sm0 = nc.alloc_sbuf_tensor("sm0", [1, NBC], F32).ap()   # chunk expsum
    mnb = nc.alloc_sbuf_tensor("mnb", [1, BC], F32).ap()    # min_c nm = -m_b
    ed0 = nc.alloc_sbuf_tensor("ed0", [1, NBC], F32).ap()   # exp(mx_c-m_b)
    w0 = nc.alloc_sbuf_tensor("w0", [1, NBC], F32).ap()
    zb = nc.alloc_sbuf_tensor("zb", [1, BC], F32).ap()
    rz = nc.alloc_sbuf_tensor("rz", [1, BC], F32).ap()
    al4 = nc.alloc_sbuf_tensor("al4", [BC, NCH], F32).ap()

    # PSUM: 4 banks A + 3 banks e = 7 of 8
    ps_a = nc.alloc_psum_tensor("psA", [128, NK * CH], F32).ap()
    ps_e = [nc.alloc_psum_tensor(f"psE{i}", [128, CH], F32).ap()
            for i in range(NEB)]

    dma_set = nc.alloc_semaphore("dma_set")
    dma_ln = [nc.alloc_semaphore(f"dma_ln{k}") for k in range(NLANE)]
    dma_g = nc.alloc_semaphore("dma_g")
    dma_out = nc.alloc_semaphore("dma_out")
    pe_mm = nc.alloc_semaphore("pe_mm")      # +1 per kt MM-group (4/step)
    pe_red = nc.alloc_semaphore("pe_red")    # +1 per step e-reduce
    act_y = nc.alloc_semaphore("act_y")      # +1 per Y half (2/step)
    act_exp = nc.alloc_semaphore("act_exp")  # +1 per chunk exp
    act_ed = nc.alloc_semaphore("act_ed")    # +1 per batch ed
    act_fin = nc.alloc_semaphore("act_fin")
    dve_p = nc.alloc_semaphore("dve_p")      # +1 per step P01 mul
    dve_s = nc.alloc_semaphore("dve_s")      # +1 per stt (2/step)
    dve_sd = nc.alloc_semaphore("dve_sd")    # DVE col-0 seed self-sync
    dve_mx = nc.alloc_semaphore("dve_mx")    # +1 per chunk max
    dve_tl = nc.alloc_semaphore("dve_tl")    # +1 per batch mnb
    dve_tc = nc.alloc_semaphore("dve_tc")    # tail chain self-sync counter
    dve_al = nc.alloc_semaphore("dve_al")    # +1 alphas ready
    dve_fin = nc.alloc_semaphore("dve_fin")
    act_sd = nc.alloc_semaphore("act_sd")    # ACT col-0 seed self-sync
    pool_t2 = nc.alloc_semaphore("pool_t2")  # +2 per step (Q1, Q2)
    pool_q = nc.alloc_semaphore("pool_q")    # +1 per step Q done

    EXP = mybir.ActivationFunctionType.Exp

    with nc.Block() as blk:
        # --- SP: all DMAs ---
        @blk.sync
        def _(sync):
            # first chunk + M first: they gate the first matmul
            sync.dma_start(enc_sb[0][:], enc_c[0]).then_inc(dma_ln[0], 16)
            setup = [
                (m_sb[:], m_p[:]),
                (ht_sb[:], h_t[:]),
                (ha_sb[:], ha_p[:]),
                (oh_sb[:], one_h[:]),
                (of_sb[:], one_f[:]),
                *[(emb_sb[b][:], emb_a[3 * b:3 * b + 3, :])
                  for b in range(BC)],
            ]
            for dst, src in setup:
                sync.dma_start(dst, src).then_inc(dma_set, 16)
            for bc in range(1, NBC):
                if bc >= NSLOT:
                    p = bc - NSLOT
                    sync.wait_ge(pe_mm, 4 * p + 4)
                    sync.wait_ge(dve_p, p + 1)
                    sync.wait_ge(dve_s, 2 * p + 2)
                if bc >= NLANE:
                    sync.wait_ge(dma_ln[bc % NLANE], 16 * (bc // NLANE))
                sync.dma_start(enc_sb[bc % NSLOT][:], enc_c[bc]) \
                    .then_inc(dma_ln[bc % NLANE], 16)
            # gather strided batch rows to contiguous [4, .] for the rescale
            sync.wait_ge(act_exp, NBC)
            sync.dma_start(ex4[:], ex_w[0:128:32, :]).then_inc(dma_g, 16)
            sync.wait_ge(dve_al, 1)
            sync.dma_start(al4[:], aw[0:128:32, :]).then_inc(dma_g, 16)
            sync.wait_ge(dve_fin, 1)
            sync.wait_ge(act_fin, 1)
            sync.dma_start(out[:], o4[:]).then_inc(dma_out, 16)
            sync.wait_ge(dma_out, 16)

        # --- PE ---
        @blk.tensor
        def _(tensor):
            def reduce(j):
                b, c, r = j // NCH, j % NCH, j % NEB
                tensor.wait_ge(pool_q, j + 1)
                if j >= NEB:
                    tensor.wait_ge(act_exp, j - NEB + 1)  # WAR ps_e[r]
                nc.tensor.matmul(
                    ps_e[r][0:1, 0:CH], oh_sb[:, 0:1],
                    q_sb[j % 3][:, 2 * CH:3 * CH],
                    start=True, stop=False)
                nc.tensor.matmul(
                    ps_e[r][0:1, 0:CH], ha_sb[0:3, b:b + 1],
                    emb_sb[b][0:3, c * CH:(c + 1) * CH],
                    start=False, stop=True).then_inc(pe_red, 1)

            tensor.wait_ge(dma_set, 16 * SETUP_DMAS)
            for bc in range(NBC):
                slot = bc % NSLOT
                tensor.wait_ge(dma_ln[bc % NLANE], 16 * (bc // NLANE + 1))
                for kt in range(NK):
                    # per-bank WAR: wait only for the drain of THIS bank from
                    # the previous step, so late stt's don't stall early MMs
                    if bc >= 1:
                        if kt == 0:
                            tensor.wait_ge(act_y, 2 * bc)       # Y of bc-1
                        elif kt == 2:
                            tensor.wait_ge(dve_s, 2 * bc - 1)   # stt P2
                        elif kt == 3:
                            tensor.wait_ge(dve_s, 2 * bc)       # stt P3
                    for j in range(NK):
                        mm = nc.tensor.matmul(
                            ps_a[:, kt * CH:(kt + 1) * CH],
                            m_sb[:, j * H + kt * 128:j * H + (kt + 1) * 128],
                            enc_sb[slot][:, j * CW:j * CW + CH],
                            start=(j == 0), stop=(j == NK - 1))
                    mm.then_inc(pe_mm, 1)
                # lag-2 reduce: gives the DVE->POOL Q chain a full extra step
                # of slack so pool_q never stalls the PE
                if bc >= 2:
                    reduce(bc - 2)
            reduce(NBC - 2)
            reduce(NBC - 1)

        # --- ACT: Y copies (h-fold, psum->sbuf fp16), chunk exp, batch ed ---
        @blk.scalar
        def _(scalar):
            def exp_op(j):
                b, c, r = j // NCH, j % NCH, j % NEB
                scalar.wait_ge(pe_red, j + 1)
                scalar.wait_ge(dve_mx, j + 1)
                nc.scalar.activation(
                    ex_w[32 * b:32 * b + 1, c * CH:(c + 1) * CH],
                    ps_e[r][0:1, 0:CH],
                    EXP, bias=nm0[0:1, j:j + 1],
                    accum_out=sm0[0:1, j:j + 1]).then_inc(act_exp, 1)

            def ed_op(b):
                scalar.wait_ge(dve_tl, b + 1)
                nc.scalar.activation(
                    ed0[0:1, NCH * b:NCH * (b + 1)],
                    nm0[0:1, NCH * b:NCH * (b + 1)],
                    EXP, bias=mnb[0:1, b:b + 1], scale=-1.0) \
                    .then_inc(act_ed, 1)

            scalar.wait_ge(dma_set, 16 * SETUP_DMAS)
            n_sd = 0
            for bc in range(NBC):
                b, c, par = bc // NCH, bc % NCH, bc % 2
                scalar.wait_ge(pe_mm, 4 * bc + 1)
                if bc >= 2:
                    scalar.wait_ge(dve_p, bc - 1)    # WAR y_sb[par]
                if c == 0:
                    nc.scalar.copy(ps_a[:, 0:1], of_sb[:]) \
                        .then_inc(act_sd, 1)
                    n_sd += 1
                    scalar.wait_ge(act_sd, n_sd)
                nc.scalar.mul(y_sb[par][:, 0:CH], ps_a[:, 0:CH],
                              ht_sb[:, NK * b:NK * b + 1]).then_inc(act_y, 1)
                scalar.wait_ge(pe_mm, 4 * bc + 2)
                if c == 0:
                    nc.scalar.copy(ps_a[:, CH:CH + 1], of_sb[:]) \
                        .then_inc(act_sd, 1)
                    n_sd += 1
                    scalar.wait_ge(act_sd, n_sd)
                nc.scalar.mul(y_sb[par][:, CH:2 * CH], ps_a[:, CH:2 * CH],
                              ht_sb[:, NK * b + 1:NK * b + 2]) \
                    .then_inc(act_y, 1)
                # lag-3 exp: pe_red/dve_mx for step j land during step j+2,
                # so an earlier exp would stall ACT and delay the next Y pair
                if bc >= 3:
                    exp_op(bc - 3)
                if bc % NCH == 3 and bc > NCH:
                    ed_op(bc // NCH - 1)
            exp_op(NBC - 3)
            exp_op(NBC - 2)
            exp_op(NBC - 1)
            ed_op(BC - 1)
            # rescale chunks 4-7
            scalar.wait_ge(dma_g, 32)
            for cc in range(4, NCH):
                op = nc.scalar.mul(o4[:, cc * CH:(cc + 1) * CH],
                                   ex4[:, cc * CH:(cc + 1) * CH],
                                   al4[0:BC, cc:cc + 1])
            op.then_inc(act_fin, 1)

        # --- DVE: P01 mul, Q tree, chunk max, tail combine, rescale ---
        @blk.vector
        def _(vector):
            def mx_op(j):
                r = j % NEB
                vector.wait_ge(pe_red, j + 1)
                nc.vector.tensor_reduce(
                    nm0[0:1, j:j + 1], ps_e[r][0:1, 0:CH],
                    mybir.AxisListType.X, mybir.AluOpType.max,
                    negate=True).then_inc(dve_mx, 1)

            n_tc = 0

            def mnb_op(b):
                vector.wait_ge(dve_mx, NCH * (b + 1))  # own nm0 writes acked
                nc.vector.tensor_reduce(
                    mnb[0:1, b:b + 1], nm0[0:1, NCH * b:NCH * (b + 1)],
                    mybir.AxisListType.X, mybir.AluOpType.min) \
                    .then_inc(dve_tl, 1)

            def wz_op(b):
                nonlocal n_tc
                vector.wait_ge(act_ed, b + 1)
                vector.wait_ge(act_exp, NCH * (b + 1))
                nc.vector.tensor_mul(w0[0:1, NCH * b:NCH * (b + 1)],
                                     ed0[0:1, NCH * b:NCH * (b + 1)],
                                     sm0[0:1, NCH * b:NCH * (b + 1)]) \
                    .then_inc(dve_tc, 1)
                n_tc += 1
                vector.wait_ge(dve_tc, n_tc)
                nc.vector.tensor_reduce(
                    zb[0:1, b:b + 1], w0[0:1, NCH * b:NCH * (b + 1)],
                    mybir.AxisListType.X, mybir.AluOpType.add) \
                    .then_inc(dve_tc, 1)
                n_tc += 1

            n_sd = 0
            for bc in range(NBC):
                b, c, par, slot = bc // NCH, bc % NCH, bc % 2, bc % NSLOT
                # P01 = Y01 * E01
                vector.wait_ge(act_y, 2 * bc + 2)
                if bc >= 2:
                    vector.wait_ge(pool_q, bc - 1)   # WAR p_sb[par]
                nc.vector.tensor_mul(
                    p_sb[par].rearrange("p (k s) -> p k s", k=NK)[:, 0:2, :],
                    y_sb[par].rearrange("p (k s) -> p k s", k=2)[:, :, :],
                    enc_sb[slot].rearrange("p (k w) -> p k w", k=NK)
                    [:, 0:2, 1:CW]).then_inc(dve_p, 1)
                # P2, P3 stt folds (psum)
                for kt in (2, 3):
                    vector.wait_ge(pe_mm, 4 * bc + kt + 1)
                    if c == 0:
                        nc.vector.tensor_copy(
                            ps_a[:, kt * CH:kt * CH + 1], of_sb[:]) \
                            .then_inc(dve_sd, 1)
                        n_sd += 1
                        vector.wait_ge(dve_sd, n_sd)
                    nc.vector.scalar_tensor_tensor(
                        p_sb[par][:, kt * CH:(kt + 1) * CH],
                        ps_a[:, kt * CH:(kt + 1) * CH],
                        ht_sb[:, NK * b + kt:NK * b + kt + 1],
                        enc_sb[slot][:, kt * CW + 1:kt * CW + CW],
                        mybir.AluOpType.mult, mybir.AluOpType.mult) \
                        .then_inc(dve_s, 1)
                if bc >= 2:
                    mx_op(bc - 2)
                if bc % NCH == 2 and bc > NCH:
                    mnb_op(bc // NCH - 1)
                if bc % NCH == 3 and bc > NCH:
                    wz_op(bc // NCH - 1)
            mx_op(NBC - 2)
            mx_op(NBC - 1)
            mnb_op(BC - 1)
            wz_op(BC - 1)
            vector.wait_ge(dve_tc, n_tc)             # zb writes acked
            nc.vector.reciprocal(rz[0:1, 0:BC], zb[0:1, 0:BC]) \
                .then_inc(dve_tc, 1)
            n_tc += 1
            vector.wait_ge(dve_tc, n_tc)             # rz write acked
            for b in range(BC):
                op = nc.vector.tensor_scalar_mul(
                    aw[32 * b:32 * b + 1, 0:NCH],
                    ed0[0:1, NCH * b:NCH * (b + 1)], rz[0:1, b:b + 1])
            op.then_inc(dve_al, 1)
            # rescale chunks 0-3
            vector.wait_ge(dma_g, 32)
            for cc in range(4):
                op = nc.vector.tensor_scalar_mul(
                    o4[:, cc * CH:(cc + 1) * CH],
                    ex4[:, cc * CH:(cc + 1) * CH], al4[0:BC, cc:cc + 1])
            op.then_inc(dve_fin, 1)

        # --- POOL (gpsimd): P2,P3 stt folds, rescale chunks 6,7 ---
        @blk.gpsimd
        def _(gpsimd):
            for bc in range(NBC):
                par, qar = bc % 2, bc % 3
                if bc >= 3:
                    gpsimd.wait_ge(pe_red, bc - 2)   # WAR q_sb[qar] (Q slice)
                gpsimd.wait_ge(dve_p, bc + 1)        # P01 landed
                nc.gpsimd.tensor_add(q_sb[qar][:, 0:CH],
                                     p_sb[par][:, 0:CH],
                                     p_sb[par][:, CH:2 * CH]) \
                    .then_inc(pool_t2, 1)
                gpsimd.wait_ge(dve_s, 2 * bc + 2)    # P2, P3 landed
                nc.gpsimd.tensor_add(q_sb[qar][:, CH:2 * CH],
                                     p_sb[par][:, 2 * CH:3 * CH],
                                     p_sb[par][:, 3 * CH:4 * CH]) \
                    .then_inc(pool_t2, 1)
                gpsimd.wait_ge(pool_t2, 2 * bc + 2)  # own writes acked
                nc.gpsimd.tensor_add(q_sb[qar][:, 2 * CH:3 * CH],
                                     q_sb[qar][:, 0:CH],
                                     q_sb[qar][:, CH:2 * CH]) \
                    .then_inc(pool_q, 1)

    return nc


def _shard_host(hidden, encoder_outputs, embedding, bigram_matrix, affect_matrix):
    """Per-core input maps. Layout/cast prep only (plus tiny h@affect)."""
    h = np.asarray(hidden, dtype=np.float32)[0]              # [B, H]
    enc = np.asarray(encoder_outputs, dtype=np.float32)      # [S, B, H]
    emb = np.asarray(embedding, dtype=np.float32)            # [S, B, 3]
    m = np.asarray(bigram_matrix, dtype=np.float32)
    aff = np.asarray(affect_matrix, dtype=np.float32)        # [H, 3]

    # padded fp16 enc: row 0 is the s=-1 halo for c==0 (value irrelevant;
    # psum col 0 is re-seeded on device)
    encp = np.zeros((S + 1, B, H), dtype=np.float16)
    encp[1:] = enc.astype(np.float16)

    m16 = m.astype(np.float16)
    m_p = np.ascontiguousarray(
        m16.reshape(NK, 128, H).transpose(1, 0, 2).reshape(128, NK * H))
    ha = (h @ aff).T.astype(np.float16)                      # [3, B]
    emb16 = np.ascontiguousarray(
        emb.transpose(1, 2, 0).astype(np.float16))           # [B, 3, S]
    one_h = np.ones((128, 1), dtype=np.float16)
    one_f = np.ones((128, 1), dtype=np.float32)

    in_maps = []
    for co in range(NCORES):
        b0 = co * BC
        # enc_c[b*NCH+c, p, k*CW+w] = encp[c*CH+w, b0+b, k*128+p]
        blocks = []
        for b in range(b0, b0 + BC):
            v = np.ascontiguousarray(encp[:, b, :])          # [S+1, H]
            w = np.lib.stride_tricks.as_strided(
                v, shape=(NCH, CW, H),
                strides=(CH * v.strides[0], v.strides[0], v.strides[1]))
            t = w.transpose(0, 2, 1).reshape(NCH, NK, 128, CW)
            blocks.append(t.transpose(0, 2, 1, 3).reshape(NCH, 128, NK * CW))
        enc_cc = np.ascontiguousarray(np.concatenate(blocks, axis=0))
        h_sl = h[b0:b0 + BC]                                 # [BC, H]
        ht = np.ascontiguousarray(
            h_sl.reshape(BC, NK, 128).transpose(2, 0, 1).reshape(128, BC * NK))
        in_maps.append({
            "enc_c": enc_cc,
            "m_p": m_p,
            "h_t": ht,
            "ha_p": np.ascontiguousarray(ha[:, b0:b0 + BC]),
            "one_h": one_h,
            "one_f": one_f,
            "emb_a": emb16[b0:b0 + BC].reshape(3 * BC, S),
        })
    return in_maps


def kernel(hidden, encoder_outputs, embedding, bigram_matrix, affect_matrix,
           _want_results=False, _spmd_kwargs=None):
    nc = _build()
    in_maps = _shard_host(hidden, encoder_outputs, embedding,
                          bigram_matrix, affect_matrix)
    res = run_bass_kernel_spmd(nc, in_maps, core_ids=list(range(NCORES)),
                               **(_spmd_kwargs or {}))
    outp = np.empty((B, 1, S), dtype=np.float32)
    for co in range(NCORES):
        outp[co * BC:(co + 1) * BC, 0, :] = res.results[co]["out"]
    if _want_results:
        return outp, res
    return outp


# revision 50
# speedup vs baseline: 1.2424x; 1.0161x over previous
"""BigramAttn Trainium2 kernel (8-core SPMD, raw Bass) — fp16 pipeline.

Reference computation (per batch b):
  e[0]   = sum_k enc[0,k] * h[k]
  e[s]   = sum_k (enc[s-1,:] @ M)[k] * h[k] * enc[s,k]          (s >= 1)
  e[s]  += sum_{k<3} (h @ affect)[k] * emb[s,k]
  out    = softmax(e)                                            # over s

Sharding: data-parallel over batch B=32 across 8 cores (4 batches/core).

fp16 data path (measured end-to-end rel err ~4e-3 vs the 2e-2 gate):
enc/M/emb/ha ship as fp16 (halves HBM traffic vs fp32; total ~17.5MB/core),
all matmuls are fp16 in / fp32 psum accumulate. h stays fp32 and is folded
on device (scalar_tensor_tensor per-partition scalar), so M is a single
shared 512KB load instead of per-batch M*diag(h) copies.

Per chunk-step (b, c) over a 513-wide enc window (1-col halo, host-packed
contiguous so each step is ONE 525KB DMA):
  PE:   A_kt[k,t] = sum_j M^T enc_prev      (16 fp16 MMs -> ps_a, 4 banks)
  ACT:  Y_01 = fp16(h_01 * A_01)            (copy+scale psum->sbuf)
  DVE:  P01  = Y01 * enc_01                 (fp16 TT, 2x mode)
  DVE:  P2,P3 = (A_kt*h_kt)*enc_kt          (stt, psum 1x; GPSIMD can't
                                             touch PSUM on trn2)
  POOL: Q = (P0+P1) + (P2+P3)               (fp16 add tree, SBUF only)
  PE:   e = ones^T Q + ha^T emb_c           (2 accumulating MMs -> ps_e)
  DVE:  nm_c = -max(e)                      (per-chunk max)
  ACT:  ex_c = exp(e + nm_c), sm_c = sum    (flash-style, overlapped)
Tail per batch: m_b = max_c mx_c; ed_c = exp(mx_c-m_b); Z_b = sum ed_c*sm_c;
alpha_c = ed_c/Z_b; out = ex_c * alpha_c (rescale split DVE/ACT/POOL).

This walrus build accepts exactly ONE semaphore wait per instruction, so the
kernel is raw Bass: per-engine programs, counting semaphores, standalone
waits. DMA completions may reorder across transfers, so chunk DMAs chain on
per-lane semaphores.
"""

import functools

import numpy as np

import concourse.bass as bass
from concourse import mybir
from concourse.bass_utils import run_bass_kernel_spmd

S, B, H = 4096, 32, 512
NCORES = 8
BC = B // NCORES          # batches per core = 4
NK = H // 128             # h-chunks = 4
CH = 512                  # s-chunk width
CW = CH + 1               # packed chunk block width (1-col halo)
NCH = S // CH             # s-chunks per batch = 8
NBC = BC * NCH            # chunk-steps per core = 32
NSLOT = 4                 # enc chunk tiles in flight
NLANE = 4                 # DMA completion-ordering lanes
NEB = 3                   # ps_e rotation depth
SETUP_DMAS = 9            # m, ht, ha, ones16, ones32, 4x emb

F32 = mybir.dt.float32
F16 = mybir.dt.float16


@functools.lru_cache(maxsize=1)
def _build():
    nc = bass.Bass("TRN2", target_bir_lowering=False, debug=False)

    enc_c = nc.dram_tensor("enc_c", [NBC, 128, NK * CW], F16,
                           kind="ExternalInput").ap()
    m_p = nc.dram_tensor("m_p", [128, NK * H], F16, kind="ExternalInput").ap()
    h_t = nc.dram_tensor("h_t", [128, BC * NK], F32, kind="ExternalInput").ap()
    ha_p = nc.dram_tensor("ha_p", [3, BC], F16, kind="ExternalInput").ap()
    one_h = nc.dram_tensor("one_h", [128, 1], F16, kind="ExternalInput").ap()
    one_f = nc.dram_tensor("one_f", [128, 1], F32, kind="ExternalInput").ap()
    emb_a = nc.dram_tensor("emb_a", [3 * BC, S], F16, kind="ExternalInput").ap()
    out = nc.dram_tensor("out", [BC, S], F32, kind="ExternalOutput").ap()

    # SBUF
    enc_sb = [nc.alloc_sbuf_tensor(f"enc{i}", [128, NK * CW], F16).ap()
              for i in range(NSLOT)]
    m_sb = nc.alloc_sbuf_tensor("m", [128, NK * H], F16).ap()
    ht_sb = nc.alloc_sbuf_tensor("ht", [128, BC * NK], F32).ap()
    ha_sb = nc.alloc_sbuf_tensor("ha", [3, BC], F16).ap()
    oh_sb = nc.alloc_sbuf_tensor("oh", [128, 1], F16).ap()
    of_sb = nc.alloc_sbuf_tensor("of", [128, 1], F32).ap()
    emb_sb = [nc.alloc_sbuf_tensor(f"emb{b}", [3, S], F16).ap()
              for b in range(BC)]
    y_sb = [nc.alloc_sbuf_tensor(f"y{i}", [128, 2 * CH], F16).ap()
            for i in range(2)]
    p_sb = [nc.alloc_sbuf_tensor(f"p{i}", [128, NK * CH], F16).ap()
            for i in range(2)]
    q_sb = [nc.alloc_sbuf_tensor(f"q{i}", [128, 3 * CH], F16).ap()
            for i in range(3)]
    ex_w = nc.alloc_sbuf_tensor("ex_w", [128, S], F32).ap()
    aw = nc.alloc_sbuf_tensor("aw", [128, NCH], F32).ap()
    ex4 = nc.alloc_sbuf_tensor("ex4", [BC, S], F32).ap()
    o4 = nc.alloc_sbuf_tensor("o4", [BC, S], F32).ap()
    nm0 = nc.alloc_sbuf_tensor("nm0", [1, NBC], F32).ap()   # -chunk max
    sm0 = nc.alloc_sbuf_tensor("sm0", [1, NBC], F32).ap()   # chunk expsum
    mnb = nc.alloc_sbuf_tensor("mnb", [1, BC], F32).ap()    # min_c nm = -m_b
    ed0 = nc.alloc_sbuf_tensor("ed0", [1, NBC], F32).ap()   # exp(mx_c-m_b)
    w0 = nc.alloc_sbuf_tensor("w0", [1, NBC], F32).ap()
    zb = nc.alloc_sbuf_tensor("zb", [1, BC], F32).ap()
    rz = nc.alloc_sbuf_tensor("rz", [1, BC], F32).ap()
    al4 = nc.alloc_sbuf_tensor("al4", [BC, NCH], F32).ap()

    # PSUM: 4 banks A + 3 banks e = 7 of 8
    ps_a = nc.alloc_psum_tensor("psA", [128, NK * CH], F32).ap()
    ps_e = [nc.alloc_psum_tensor(f"psE{i}", [128, CH], F32).ap()
            for i in range(NEB)]

    dma_set = nc.alloc_semaphore("dma_set")  # oh, ha, emb (reduce deps)
    dma_m = nc.alloc_semaphore("dma_m")      # m_sb (PE main dep)
    dma_h = nc.alloc_semaphore("dma_h")      # ht, of (ACT/DVE deps)
    dma_ln = [nc.alloc_semaphore(f"dma_ln{k}") for k in range(NLANE)]
    dma_g = nc.alloc_semaphore("dma_g")
    dma_out = nc.alloc_semaphore("dma_out")
    pe_mm = nc.alloc_semaphore("pe_mm")      # +1 per kt MM-group (4/step)
    pe_red = nc.alloc_semaphore("pe_red")    # +1 per step e-reduce
    act_y = nc.alloc_semaphore("act_y")      # +1 per Y half (2/step)
    act_exp = nc.alloc_semaphore("act_exp")  # +1 per chunk exp
    act_ed = nc.alloc_semaphore("act_ed")    # +1 per batch ed
    act_fin = nc.alloc_semaphore("act_fin")
    dve_p = nc.alloc_semaphore("dve_p")      # +1 per step P01 mul
    dve_s = nc.alloc_semaphore("dve_s")      # +1 per stt (2/step)
    dve_sd = nc.alloc_semaphore("dve_sd")    # DVE col-0 seed self-sync
    dve_mx = nc.alloc_semaphore("dve_mx")    # +1 per chunk max
    dve_tl = nc.alloc_semaphore("dve_tl")    # +1 per batch mnb
    dve_tc = nc.alloc_semaphore("dve_tc")    # tail chain self-sync counter
    dve_al = nc.alloc_semaphore("dve_al")    # +1 alphas ready
    dve_fin = nc.alloc_semaphore("dve_fin")
    act_sd = nc.alloc_semaphore("act_sd")    # ACT col-0 seed self-sync
    pool_t2 = nc.alloc_semaphore("pool_t2")  # +2 per step (Q1, Q2)
    pool_q = nc.alloc_semaphore("pool_q")    # +1 per step Q done

    EXP = mybir.ActivationFunctionType.Exp

    with nc.Block() as blk:
        # --- SP: all DMAs ---
        @blk.sync
        def _(sync):
            # first chunk + M first: they gate the first matmul. Setup DMAs
            # use per-consumer-group semaphores (completions can reorder, so
            # partial-sum thresholds on one semaphore would be ambiguous).
            sync.dma_start(enc_sb[0][:], enc_c[0]).then_inc(dma_ln[0], 16)
            sync.dma_start(m_sb[:], m_p[:]).then_inc(dma_m, 16)
            sync.dma_start(ht_sb[:], h_t[:]).then_inc(dma_h, 16)
            sync.dma_start(of_sb[:], one_f[:]).then_inc(dma_h, 16)
            setup = [
                (oh_sb[:], one_h[:]),
                (ha_sb[:], ha_p[:]),
                *[(emb_sb[b][:], emb_a[3 * b:3 * b + 3, :])
                  for b in range(BC)],
            ]
            for dst, src in setup:
                sync.dma_start(dst, src).then_inc(dma_set, 16)
            for bc in range(1, NBC):
                if bc >= NSLOT:
                    p = bc - NSLOT
                    sync.wait_ge(pe_mm, 4 * p + 4)
                    sync.wait_ge(dve_p, p + 1)
                    sync.wait_ge(dve_s, 2 * p + 2)
                if bc >= NLANE:
                    sync.wait_ge(dma_ln[bc % NLANE], 16 * (bc // NLANE))
                sync.dma_start(enc_sb[bc % NSLOT][:], enc_c[bc]) \
                    .then_inc(dma_ln[bc % NLANE], 16)
            # gather ex_w batch rows chunk-by-chunk as their exps complete
            for cc in range(NCH):
                sync.wait_ge(act_exp, 3 * NCH + cc + 1)  # exp(24+cc) done
                sync.dma_start(ex4[:, cc * CH:(cc + 1) * CH],
                               ex_w[0:128:32, cc * CH:(cc + 1) * CH]) \
                    .then_inc(dma_g, 16)
            sync.wait_ge(dve_al, 1)
            sync.dma_start(al4[:], aw[0:128:32, :]).then_inc(dma_g, 16)
            sync.wait_ge(dve_fin, 1)
            sync.wait_ge(act_fin, 1)
            # 16 lines of 4KB so the write spreads across all DMA engines
            sync.dma_start(out.rearrange("b (q w) -> (b q) w", q=4),
                           o4.rearrange("p (q w) -> p q w", q=4)) \
                .then_inc(dma_out, 16)
            sync.wait_ge(dma_out, 16)

        # --- PE ---
        @blk.tensor
        def _(tensor):
            def reduce(j):
                b, c, r = j // NCH, j % NCH, j % NEB
                if j == 0:
                    tensor.wait_ge(dma_set, 96)  # oh, ha, emb
                tensor.wait_ge(pool_q, j + 1)
                if j >= NEB:
                    tensor.wait_ge(act_exp, j - NEB + 1)  # WAR ps_e[r]
                nc.tensor.matmul(
                    ps_e[r][0:1, 0:CH], oh_sb[:, 0:1],
                    q_sb[j % 3][:, 2 * CH:3 * CH],
                    start=True, stop=False)
                nc.tensor.matmul(
                    ps_e[r][0:1, 0:CH], ha_sb[0:3, b:b + 1],
                    emb_sb[b][0:3, c * CH:(c + 1) * CH],
                    start=False, stop=True).then_inc(pe_red, 1)

            tensor.wait_ge(dma_m, 16)            # m_sb only
            for bc in range(NBC):
                slot = bc % NSLOT
                tensor.wait_ge(dma_ln[bc % NLANE], 16 * (bc // NLANE + 1))
                for kt in range(NK):
                    # per-bank WAR: wait only for the drain of THIS bank from
                    # the previous step, so late stt's don't stall early MMs
                    if bc >= 1:
                        if kt == 0:
                            tensor.wait_ge(act_y, 2 * bc)       # Y of bc-1
                        elif kt == 2:
                            tensor.wait_ge(dve_s, 2 * bc - 1)   # stt P2
                        elif kt == 3:
                            tensor.wait_ge(dve_s, 2 * bc)       # stt P3
                    for j in range(NK):
                        mm = nc.tensor.matmul(
                            ps_a[:, kt * CH:(kt + 1) * CH],
                            m_sb[:, j * H + kt * 128:j * H + (kt + 1) * 128],
                            enc_sb[slot][:, j * CW:j * CW + CH],
                            start=(j == 0), stop=(j == NK - 1))
                    mm.then_inc(pe_mm, 1)
                # lag-2 reduce: gives the DVE->POOL Q chain a full extra step
                # of slack so pool_q never stalls the PE
                if bc >= 2:
                    reduce(bc - 2)
            reduce(NBC - 2)
            reduce(NBC - 1)

        # --- ACT: Y copies (h-fold, psum->sbuf fp16), chunk exp, batch ed ---
        @blk.scalar
        def _(scalar):
            def exp_op(j):
                b, c, r = j // NCH, j % NCH, j % NEB
                scalar.wait_ge(pe_red, j + 1)
                scalar.wait_ge(dve_mx, j + 1)
                nc.scalar.activation(
                    ex_w[32 * b:32 * b + 1, c * CH:(c + 1) * CH],
                    ps_e[r][0:1, 0:CH],
                    EXP, bias=nm0[0:1, j:j + 1],
                    accum_out=sm0[0:1, j:j + 1]).then_inc(act_exp, 1)

            def ed_op(b):
                scalar.wait_ge(dve_tl, b + 1)
                nc.scalar.activation(
                    ed0[0:1, NCH * b:NCH * (b + 1)],
                    nm0[0:1, NCH * b:NCH * (b + 1)],
                    EXP, bias=mnb[0:1, b:b + 1], scale=-1.0) \
                    .then_inc(act_ed, 1)

            scalar.wait_ge(dma_h, 32)            # ht, of
            n_sd = 0
            for bc in range(NBC):
                b, c, par = bc // NCH, bc % NCH, bc % 2
                scalar.wait_ge(pe_mm, 4 * bc + 1)
                if bc >= 2:
                    scalar.wait_ge(dve_p, bc - 1)    # WAR y_sb[par]
                if c == 0:
                    nc.scalar.copy(ps_a[:, 0:1], of_sb[:]) \
                        .then_inc(act_sd, 1)
                    n_sd += 1
                    scalar.wait_ge(act_sd, n_sd)
                nc.scalar.mul(y_sb[par][:, 0:CH], ps_a[:, 0:CH],
                              ht_sb[:, NK * b:NK * b + 1]).then_inc(act_y, 1)
                scalar.wait_ge(pe_mm, 4 * bc + 2)
                if c == 0:
                    nc.scalar.copy(ps_a[:, CH:CH + 1], of_sb[:]) \
                        .then_inc(act_sd, 1)
                    n_sd += 1
                    scalar.wait_ge(act_sd, n_sd)
                nc.scalar.mul(y_sb[par][:, CH:2 * CH], ps_a[:, CH:2 * CH],
                              ht_sb[:, NK * b + 1:NK * b + 2]) \
                    .then_inc(act_y, 1)
                # lag-3 exp: pe_red/dve_mx for step j land during step j+2,
                # so an earlier exp would stall ACT and delay the next Y pair
                if bc >= 3:
                    exp_op(bc - 3)
                if bc % NCH == 3 and bc > NCH:
                    ed_op(bc // NCH - 1)
            exp_op(NBC - 3)
            exp_op(NBC - 2)
            exp_op(NBC - 1)
            ed_op(BC - 1)
            # rescale chunks 4-7
            scalar.wait_ge(dma_g, 16 * (NCH + 1))
            for cc in range(4, NCH):
                op = nc.scalar.mul(o4[:, cc * CH:(cc + 1) * CH],
                                   ex4[:, cc * CH:(cc + 1) * CH],
                                   al4[0:BC, cc:cc + 1])
            op.then_inc(act_fin, 1)

        # --- DVE: P01 mul, Q tree, chunk max, tail combine, rescale ---
        @blk.vector
        def _(vector):
            def mx_op(j):
                r = j % NEB
                vector.wait_ge(pe_red, j + 1)
                nc.vector.tensor_reduce(
                    nm0[0:1, j:j + 1], ps_e[r][0:1, 0:CH],
                    mybir.AxisListType.X, mybir.AluOpType.max,
                    negate=True).then_inc(dve_mx, 1)

            n_tc = 0

            def mnb_op(b):
                vector.wait_ge(dve_mx, NCH * (b + 1))  # own nm0 writes acked
                nc.vector.tensor_reduce(
                    mnb[0:1, b:b + 1], nm0[0:1, NCH * b:NCH * (b + 1)],
                    mybir.AxisListType.X, mybir.AluOpType.min) \
                    .then_inc(dve_tl, 1)

            def wz_op(b):
                nonlocal n_tc
                vector.wait_ge(act_ed, b + 1)
                vector.wait_ge(act_exp, NCH * (b + 1))
                nc.vector.tensor_mul(w0[0:1, NCH * b:NCH * (b + 1)],
                                     ed0[0:1, NCH * b:NCH * (b + 1)],
                                     sm0[0:1, NCH * b:NCH * (b + 1)]) \
                    .then_inc(dve_tc, 1)
                n_tc += 1
                vector.wait_ge(dve_tc, n_tc)
                nc.vector.tensor_reduce(
                    zb[0:1, b:b + 1], w0[0:1, NCH * b:NCH * (b + 1)],
                    mybir.AxisListType.X, mybir.AluOpType.add) \
                    .then_inc(dve_tc, 1)
                n_tc += 1

            n_sd = 0
            vector.wait_ge(dma_h, 32)            # ht, of
            for bc in range(NBC):
                b, c, par, slot = bc // NCH, bc % NCH, bc % 2, bc % NSLOT
                # P01 = Y01 * E01
                vector.wait_ge(act_y, 2 * bc + 2)
                if bc >= 2:
                    vector.wait_ge(pool_q, bc - 1)   # WAR p_sb[par]
                nc.vector.tensor_mul(
                    p_sb[par].rearrange("p (k s) -> p k s", k=NK)[:, 0:2, :],
                    y_sb[par].rearrange("p (k s) -> p k s", k=2)[:, :, :],
                    enc_sb[slot].rearrange("p (k w) -> p k w", k=NK)
                    [:, 0:2, 1:CW]).then_inc(dve_p, 1)
                # P2, P3 stt folds (psum)
                for kt in (2, 3):
                    vector.wait_ge(pe_mm, 4 * bc + kt + 1)
                    if c == 0:
                        nc.vector.tensor_copy(
                            ps_a[:, kt * CH:kt * CH + 1], of_sb[:]) \
                            .then_inc(dve_sd, 1)
                        n_sd += 1
                        vector.wait_ge(dve_sd, n_sd)
                    nc.vector.scalar_tensor_tensor(
                        p_sb[par][:, kt * CH:(kt + 1) * CH],
                        ps_a[:, kt * CH:(kt + 1) * CH],
                        ht_sb[:, NK * b + kt:NK * b + kt + 1],
                        enc_sb[slot][:, kt * CW + 1:kt * CW + CW],
                        mybir.AluOpType.mult, mybir.AluOpType.mult) \
                        .then_inc(dve_s, 1)
                if bc >= 2:
                    mx_op(bc - 2)
                if bc % NCH == 2 and bc > NCH:
                    mnb_op(bc // NCH - 1)
                if bc % NCH == 3 and bc > NCH:
                    wz_op(bc // NCH - 1)
            mx_op(NBC - 2)
            mx_op(NBC - 1)
            mnb_op(BC - 1)
            wz_op(BC - 1)
            vector.wait_ge(dve_tc, n_tc)             # zb writes acked
            nc.vector.reciprocal(rz[0:1, 0:BC], zb[0:1, 0:BC]) \
                .then_inc(dve_tc, 1)
            n_tc += 1
            vector.wait_ge(dve_tc, n_tc)             # rz write acked
            for b in range(BC):
                op = nc.vector.tensor_scalar_mul(
                    aw[32 * b:32 * b + 1, 0:NCH],
                    ed0[0:1, NCH * b:NCH * (b + 1)], rz[0:1, b:b + 1])
            op.then_inc(dve_al, 1)
            # rescale chunks 0-3
            vector.wait_ge(dma_g, 16 * (NCH + 1))
            for cc in range(4):
                op = nc.vector.tensor_scalar_mul(
                    o4[:, cc * CH:(cc + 1) * CH],
                    ex4[:, cc * CH:(cc + 1) * CH], al4[0:BC, cc:cc + 1])
            op.then_inc(dve_fin, 1)

        # --- POOL (gpsimd): P2,P3 stt folds, rescale chunks 6,7 ---
        @blk.gpsimd
        def _(gpsimd):
            for bc in range(NBC):
                par, qar = bc % 2, bc % 3
                if bc >= 3:
                    gpsimd.wait_ge(pe_red, bc - 2)   # WAR q_sb[qar] (Q slice)
                gpsimd.wait_ge(dve_p, bc + 1)        # P01 landed
                nc.gpsimd.tensor_add(q_sb[qar][:, 0:CH],
                                     p_sb[par][:, 0:CH],
                                     p_sb[par][:, CH:2 * CH]) \
                    .then_inc(pool_t2, 1)
                gpsimd.wait_ge(dve_s, 2 * bc + 2)    # P2, P3 landed
                nc.gpsimd.tensor_add(q_sb[qar][:, CH:2 * CH],
                                     p_sb[par][:, 2 * CH:3 * CH],
                                     p_sb[par][:, 3 * CH:4 * CH]) \
                    .then_inc(pool_t2, 1)
                gpsimd.wait_ge(pool_t2, 2 * bc + 2)  # own writes acked
                nc.gpsimd.tensor_add(q_sb[qar][:, 2 * CH:3 * CH],
                                     q_sb[qar][:, 0:CH],
                                     q_sb[qar][:, CH:2 * CH]) \
                    .then_inc(pool_q, 1)

    return nc


def _shard_host(hidden, encoder_outputs, embedding, bigram_matrix, affect_matrix):
    """Per-core input maps. Layout/cast prep only (plus tiny h@affect)."""
    h = np.asarray(hidden, dtype=np.float32)[0]              # [B, H]
    enc = np.asarray(encoder_outputs, dtype=np.float32)      # [S, B, H]
    emb = np.asarray(embedding, dtype=np.float32)            # [S, B, 3]
    m = np.asarray(bigram_matrix, dtype=np.float32)
    aff = np.asarray(affect_matrix, dtype=np.float32)        # [H, 3]

    # padded fp16 enc: row 0 is the s=-1 halo for c==0 (value irrelevant;
    # psum col 0 is re-seeded on device)
    encp = np.zeros((S + 1, B, H), dtype=np.float16)
    encp[1:] = enc.astype(np.float16)

    m16 = m.astype(np.float16)
    m_p = np.ascontiguousarray(
        m16.reshape(NK, 128, H).transpose(1, 0, 2).reshape(128, NK * H))
    ha = (h @ aff).T.astype(np.float16)                      # [3, B]
    emb16 = np.ascontiguousarray(
        emb.transpose(1, 2, 0).astype(np.float16))           # [B, 3, S]
    one_h = np.ones((128, 1), dtype=np.float16)
    one_f = np.ones((128, 1), dtype=np.float32)

    in_maps = []
    for co in range(NCORES):
        b0 = co * BC
        # enc_c[b*NCH+c, p, k*CW+w] = encp[c*CH+w, b0+b, k*128+p]
        blocks = []
        for b in range(b0, b0 + BC):
            v = np.ascontiguousarray(encp[:, b, :])          # [S+1, H]
            w = np.lib.stride_tricks.as_strided(
                v, shape=(NCH, CW, H),
                strides=(CH * v.strides[0], v.strides[0], v.strides[1]))
            t = w.transpose(0, 2, 1).reshape(NCH, NK, 128, CW)
            blocks.append(t.transpose(0, 2, 1, 3).reshape(NCH, 128, NK * CW))
        enc_cc = np.ascontiguousarray(np.concatenate(blocks, axis=0))
        h_sl = h[b0:b0 + BC]                                 # [BC, H]
        ht = np.ascontiguousarray(
            h_sl.reshape(BC, NK, 128).transpose(2, 0, 1).reshape(128, BC * NK))
        in_maps.append({
            "enc_c": enc_cc,
            "m_p": m_p,
            "h_t": ht,
            "ha_p": np.ascontiguousarray(ha[:, b0:b0 + BC]),
            "one_h": one_h,
            "one_f": one_f,
            "emb_a": emb16[b0:b0 + BC].reshape(3 * BC, S),
        })
    return in_maps


def kernel(hidden, encoder_outputs, embedding, bigram_matrix, affect_matrix,
           _want_results=False, _spmd_kwargs=None):
    nc = _build()
    in_maps = _shard_host(hidden, encoder_outputs, embedding,
                          bigram_matrix, affect_matrix)
    res = run_bass_kernel_spmd(nc, in_maps, core_ids=list(range(NCORES)),
                               **(_spmd_kwargs or {}))
    outp = np.empty((B, 1, S), dtype=np.float32)
    for co in range(NCORES):
        outp[co * BC:(co + 1) * BC, 0, :] = res.results[co]["out"]
    if _want_results:
        return outp, res
    return outp


# revision 59
# speedup vs baseline: 1.2736x; 1.0252x over previous
"""BigramAttn Trainium2 kernel (8-core SPMD, raw Bass) — fp16 pipeline.

Reference computation (per batch b):
  e[0]   = sum_k enc[0,k] * h[k]
  e[s]   = sum_k (enc[s-1,:] @ M)[k] * h[k] * enc[s,k]          (s >= 1)
  e[s]  += sum_{k<3} (h @ affect)[k] * emb[s,k]
  out    = softmax(e)                                            # over s

Sharding: data-parallel over batch B=32 across 8 cores (4 batches/core).

fp16 data path (measured end-to-end rel err ~4e-3 vs the 2e-2 gate):
enc/M/emb/ha ship as fp16 (halves HBM traffic vs fp32; total ~17.5MB/core),
all matmuls are fp16 in / fp32 psum accumulate. h stays fp32 and is folded
on device (scalar_tensor_tensor per-partition scalar), so M is a single
shared 512KB load instead of per-batch M*diag(h) copies.

Per chunk-step (b, c) over a 513-wide enc window (1-col halo, host-packed
contiguous so each step is ONE 525KB DMA):
  PE:   A_kt[k,t] = sum_j M^T enc_prev      (16 fp16 MMs -> ps_a, 4 banks)
  ACT:  Y_01 = fp16(h_01 * A_01)            (copy+scale psum->sbuf)
  DVE:  P01  = Y01 * enc_01                 (fp16 TT, 2x mode)
  DVE:  P2,P3 = (A_kt*h_kt)*enc_kt          (stt, psum 1x; GPSIMD can't
                                             touch PSUM on trn2)
  POOL: Q = (P0+P1) + (P2+P3)               (fp16 add tree, SBUF only)
  PE:   e = ones^T Q + ha^T emb_c           (2 accumulating MMs -> ps_e)
  DVE:  nm_c = -max(e)                      (per-chunk max)
  ACT:  ex_c = exp(e + nm_c), sm_c = sum    (flash-style, overlapped)
Tail per batch: m_b = max_c mx_c; ed_c = exp(mx_c-m_b); Z_b = sum ed_c*sm_c;
alpha_c = ed_c/Z_b; out = ex_c * alpha_c (rescale split DVE/ACT/POOL).

This walrus build accepts exactly ONE semaphore wait per instruction, so the
kernel is raw Bass: per-engine programs, counting semaphores, standalone
waits. DMA completions may reorder across transfers, so chunk DMAs chain on
per-lane semaphores.
"""

import functools

import numpy as np

import concourse.bass as bass
from concourse import mybir
from concourse.bass_utils import run_bass_kernel_spmd

S, B, H = 4096, 32, 512
NCORES = 8
BC = B // NCORES          # batches per core = 4
NK = H // 128             # h-chunks = 4
CH = 512                  # s-chunk width
CW = CH + 1               # packed chunk block width (1-col halo)
NCH = S // CH             # s-chunks per batch = 8
NBC = BC * NCH            # chunk-steps per core = 32
NSLOT = 4                 # enc chunk tiles in flight
NLANE = 4                 # DMA completion-ordering lanes
NEB = 3                   # ps_e rotation depth
SETUP_DMAS = 9            # m, ht, ha, ones16, ones32, 4x emb

F32 = mybir.dt.float32
F16 = mybir.dt.float16


@functools.lru_cache(maxsize=1)
def _build():
    nc = bass.Bass("TRN2", target_bir_lowering=False, debug=False)

    enc_c = nc.dram_tensor("enc_c", [NBC, 128, NK * CW], F16,
                           kind="ExternalInput").ap()
    m_p = nc.dram_tensor("m_p", [128, NK * H], F16, kind="ExternalInput").ap()
    h_t = nc.dram_tensor("h_t", [128, BC * NK], F32, kind="ExternalInput").ap()
    ha_p = nc.dram_tensor("ha_p", [3, BC], F16, kind="ExternalInput").ap()
    one_h = nc.dram_tensor("one_h", [128, 1], F16, kind="ExternalInput").ap()
    one_f = nc.dram_tensor("one_f", [128, 1], F32, kind="ExternalInput").ap()
    emb_a = nc.dram_tensor("emb_a", [3 * BC, S], F16, kind="ExternalInput").ap()
    out = nc.dram_tensor("out", [BC, S], F32, kind="ExternalOutput").ap()

    # SBUF
    enc_sb = [nc.alloc_sbuf_tensor(f"enc{i}", [128, NK * CW], F16).ap()
              for i in range(NSLOT)]
    m_sb = nc.alloc_sbuf_tensor("m", [128, NK * H], F16).ap()
    ht_sb = nc.alloc_sbuf_tensor("ht", [128, BC * NK], F32).ap()
    ha_sb = nc.alloc_sbuf_tensor("ha", [3, BC], F16).ap()
    oh_sb = nc.alloc_sbuf_tensor("oh", [128, 1], F16).ap()
    of_sb = nc.alloc_sbuf_tensor("of", [128, 1], F32).ap()
    emb_sb = [nc.alloc_sbuf_tensor(f"emb{b}", [3, S], F16).ap()
              for b in range(BC)]
    y_sb = [nc.alloc_sbuf_tensor(f"y{i}", [128, 2 * CH], F16).ap()
            for i in range(2)]
    p_sb = [nc.alloc_sbuf_tensor(f"p{i}", [128, NK * CH], F16).ap()
            for i in range(2)]
    q_sb = [nc.alloc_sbuf_tensor(f"q{i}", [128, 3 * CH], F16).ap()
            for i in range(3)]
    ex_w = nc.alloc_sbuf_tensor("ex_w", [128, S], F32).ap()
    aw = nc.alloc_sbuf_tensor("aw", [128, NCH], F32).ap()
    ex4 = nc.alloc_sbuf_tensor("ex4", [BC, S], F32).ap()
    o4 = nc.alloc_sbuf_tensor("o4", [BC, S], F32).ap()
    nm0 = nc.alloc_sbuf_tensor("nm0", [1, NBC], F32).ap()   # -chunk max
    sm0 = nc.alloc_sbuf_tensor("sm0", [1, NBC], F32).ap()   # chunk expsum
    mnb = nc.alloc_sbuf_tensor("mnb", [1, BC], F32).ap()    # min_c nm = -m_b
    ed0 = nc.alloc_sbuf_tensor("ed0", [1, NBC], F32).ap()   # exp(mx_c-m_b)
    w0 = nc.alloc_sbuf_tensor("w0", [1, NBC], F32).ap()
    zb = nc.alloc_sbuf_tensor("zb", [1, BC], F32).ap()
    rz = nc.alloc_sbuf_tensor("rz", [1, BC], F32).ap()
    al4 = nc.alloc_sbuf_tensor("al4", [BC, NCH], F32).ap()

    # PSUM: 4 banks A + 3 banks e = 7 of 8
    ps_a = nc.alloc_psum_tensor("psA", [128, NK * CH], F32).ap()
    ps_e = [nc.alloc_psum_tensor(f"psE{i}", [128, CH], F32).ap()
            for i in range(NEB)]

    dma_set = nc.alloc_semaphore("dma_set")  # oh, ha, emb (reduce deps)
    dma_m = nc.alloc_semaphore("dma_m")      # m_sb (PE main dep)
    dma_h = nc.alloc_semaphore("dma_h")      # ht, of (ACT/DVE deps)
    dma_ln = [nc.alloc_semaphore(f"dma_ln{k}") for k in range(NLANE)]
    dma_g = nc.alloc_semaphore("dma_g")
    dma_out = nc.alloc_semaphore("dma_out")
    pe_mm = nc.alloc_semaphore("pe_mm")      # +1 per kt MM-group (4/step)
    pe_red = nc.alloc_semaphore("pe_red")    # +1 per step e-reduce
    act_y = nc.alloc_semaphore("act_y")      # +1 per Y half (2/step)
    act_exp = nc.alloc_semaphore("act_exp")  # +1 per chunk exp
    act_ed = nc.alloc_semaphore("act_ed")    # +1 per batch ed
    act_fin = nc.alloc_semaphore("act_fin")
    dve_p = nc.alloc_semaphore("dve_p")      # +1 per step P01 mul
    dve_s = nc.alloc_semaphore("dve_s")      # +1 per stt (2/step)
    dve_sd = nc.alloc_semaphore("dve_sd")    # DVE col-0 seed self-sync
    dve_mx = nc.alloc_semaphore("dve_mx")    # +1 per chunk max
    dve_tl = nc.alloc_semaphore("dve_tl")    # +1 per batch mnb
    dve_tc = nc.alloc_semaphore("dve_tc")    # tail chain self-sync counter
    dve_al = nc.alloc_semaphore("dve_al")    # +1 alphas ready
    dve_fin = nc.alloc_semaphore("dve_fin")
    act_sd = nc.alloc_semaphore("act_sd")    # ACT col-0 seed self-sync
    pool_t2 = nc.alloc_semaphore("pool_t2")  # +2 per step (Q1, Q2)
    pool_q = nc.alloc_semaphore("pool_q")    # +1 per step Q done
    dve_qt = nc.alloc_semaphore("dve_qt")    # endgame DVE Q1/Q2 self-sync
    dve_q2 = nc.alloc_semaphore("dve_q2")    # endgame DVE Q done

    EXP = mybir.ActivationFunctionType.Exp

    with nc.Block() as blk:
        # --- SP: all DMAs ---
        @blk.sync
        def _(sync):
            # first chunk first: it gates the first matmul (m/ht/of go out
            # in parallel on ACT's HWDGE queue). Setup DMAs use
            # per-consumer-group semaphores (completions can reorder, so
            # partial-sum thresholds on one semaphore would be ambiguous).
            sync.dma_start(enc_sb[0][:], enc_c[0]).then_inc(dma_ln[0], 16)
            setup = [
                (oh_sb[:], one_h[:]),
                (ha_sb[:], ha_p[:]),
                *[(emb_sb[b][:], emb_a[3 * b:3 * b + 3, :])
                  for b in range(BC)],
            ]
            for dst, src in setup:
                sync.dma_start(dst, src).then_inc(dma_set, 16)
            for bc in range(1, NBC):
                if bc >= NSLOT:
                    p = bc - NSLOT
                    sync.wait_ge(pe_mm, 4 * p + 4)
                    sync.wait_ge(dve_p, p + 1)
                    sync.wait_ge(dve_s, 2 * p + 2)
                if bc >= NLANE:
                    sync.wait_ge(dma_ln[bc % NLANE], 16 * (bc // NLANE))
                sync.dma_start(enc_sb[bc % NSLOT][:], enc_c[bc]) \
                    .then_inc(dma_ln[bc % NLANE], 16)
            # gather ex_w batch rows chunk-by-chunk as their exps complete
            for cc in range(NCH):
                sync.wait_ge(act_exp, 3 * NCH + cc + 1)  # exp(24+cc) done
                sync.dma_start(ex4[:, cc * CH:(cc + 1) * CH],
                               ex_w[0:128:32, cc * CH:(cc + 1) * CH]) \
                    .then_inc(dma_g, 16)
            sync.wait_ge(dve_al, 1)
            sync.dma_start(al4[:], aw[0:128:32, :]).then_inc(dma_g, 16)
            # out in 2 pieces (gated on each rescale half), many 2KB lines
            # so the writes spread across DMA engines and start early
            sync.wait_ge(dve_fin, 1)
            sync.dma_start(
                out[:, 0:4 * CH].rearrange("b (q w) -> b q w", q=4),
                o4[:, 0:4 * CH].rearrange("p (q w) -> p q w", q=4)) \
                .then_inc(dma_out, 16)
            sync.wait_ge(act_fin, 1)
            sync.dma_start(
                out[:, 4 * CH:S].rearrange("b (q w) -> b q w", q=4),
                o4[:, 4 * CH:S].rearrange("p (q w) -> p q w", q=4)) \
                .then_inc(dma_out, 16)
            sync.wait_ge(dma_out, 32)

        # --- PE ---
        @blk.tensor
        def _(tensor):
            def reduce(j):
                b, c, r = j // NCH, j % NCH, j % NEB
                if j == 0:
                    tensor.wait_ge(dma_set, 96)  # oh, ha, emb
                if j >= NBC - 2:
                    tensor.wait_ge(dve_q2, j - (NBC - 2) + 1)
                else:
                    tensor.wait_ge(pool_q, j + 1)
                if j >= NEB:
                    tensor.wait_ge(act_exp, j - NEB + 1)  # WAR ps_e[r]
                nc.tensor.matmul(
                    ps_e[r][0:1, 0:CH], oh_sb[:, 0:1],
                    q_sb[j % 3][:, 2 * CH:3 * CH],
                    start=True, stop=False)
                nc.tensor.matmul(
                    ps_e[r][0:1, 0:CH], ha_sb[0:3, b:b + 1],
                    emb_sb[b][0:3, c * CH:(c + 1) * CH],
                    start=False, stop=True).then_inc(pe_red, 1)

            tensor.wait_ge(dma_m, 16)            # m_sb only
            for bc in range(NBC):
                slot = bc % NSLOT
                tensor.wait_ge(dma_ln[bc % NLANE], 16 * (bc // NLANE + 1))
                for kt in range(NK):
                    # per-bank WAR: wait only for the drain of THIS bank from
                    # the previous step, so late stt's don't stall early MMs
                    if bc >= 1:
                        if kt == 0:
                            tensor.wait_ge(act_y, 2 * bc)       # Y of bc-1
                        elif kt == 2:
                            tensor.wait_ge(dve_s, 2 * bc - 1)   # stt P2
                        elif kt == 3:
                            tensor.wait_ge(dve_s, 2 * bc)       # stt P3
                    for j in range(NK):
                        mm = nc.tensor.matmul(
                            ps_a[:, kt * CH:(kt + 1) * CH],
                            m_sb[:, j * H + kt * 128:j * H + (kt + 1) * 128],
                            enc_sb[slot][:, j * CW:j * CW + CH],
                            start=(j == 0), stop=(j == NK - 1))
                    mm.then_inc(pe_mm, 1)
                # lag-2 reduce: gives the DVE->POOL Q chain a full extra step
                # of slack so pool_q never stalls the PE
                if bc >= 2:
                    reduce(bc - 2)
            reduce(NBC - 2)
            reduce(NBC - 1)

        # --- ACT: Y copies (h-fold, psum->sbuf fp16), chunk exp, batch ed ---
        @blk.scalar
        def _(scalar):
            def exp_op(j):
                b, c, r = j // NCH, j % NCH, j % NEB
                scalar.wait_ge(pe_red, j + 1)
                scalar.wait_ge(dve_mx, j + 1)
                nc.scalar.activation(
                    ex_w[32 * b:32 * b + 1, c * CH:(c + 1) * CH],
                    ps_e[r][0:1, 0:CH],
                    EXP, bias=nm0[0:1, j:j + 1],
                    accum_out=sm0[0:1, j:j + 1]).then_inc(act_exp, 1)

            def ed_op(b):
                scalar.wait_ge(dve_tl, b + 1)
                nc.scalar.activation(
                    ed0[0:1, NCH * b:NCH * (b + 1)],
                    nm0[0:1, NCH * b:NCH * (b + 1)],
                    EXP, bias=mnb[0:1, b:b + 1], scale=-1.0) \
                    .then_inc(act_ed, 1)

            # issue m/ht/of on ACT's own HWDGE queue, parallel to SP's chunk0
            scalar.dma_start(m_sb[:], m_p[:]).then_inc(dma_m, 16)
            scalar.dma_start(ht_sb[:], h_t[:]).then_inc(dma_h, 16)
            scalar.dma_start(of_sb[:], one_f[:]).then_inc(dma_h, 16)
            scalar.wait_ge(dma_h, 32)            # ht, of
            n_sd = 0
            for bc in range(NBC):
                b, c, par = bc // NCH, bc % NCH, bc % 2
                scalar.wait_ge(pe_mm, 4 * bc + 1)
                if bc >= 2:
                    scalar.wait_ge(dve_p, bc - 1)    # WAR y_sb[par]
                if c == 0:
                    nc.scalar.copy(ps_a[:, 0:1], of_sb[:]) \
                        .then_inc(act_sd, 1)
                    n_sd += 1
                    scalar.wait_ge(act_sd, n_sd)
                nc.scalar.mul(y_sb[par][:, 0:CH], ps_a[:, 0:CH],
                              ht_sb[:, NK * b:NK * b + 1]).then_inc(act_y, 1)
                scalar.wait_ge(pe_mm, 4 * bc + 2)
                if c == 0:
                    nc.scalar.copy(ps_a[:, CH:CH + 1], of_sb[:]) \
                        .then_inc(act_sd, 1)
                    n_sd += 1
                    scalar.wait_ge(act_sd, n_sd)
                nc.scalar.mul(y_sb[par][:, CH:2 * CH], ps_a[:, CH:2 * CH],
                              ht_sb[:, NK * b + 1:NK * b + 2]) \
                    .then_inc(act_y, 1)
                # lag-3 exp: pe_red/dve_mx for step j land during step j+2,
                # so an earlier exp would stall ACT and delay the next Y pair
                if bc >= 3:
                    exp_op(bc - 3)
                if bc % NCH == 3 and bc > NCH:
                    ed_op(bc // NCH - 1)
            exp_op(NBC - 3)
            exp_op(NBC - 2)
            exp_op(NBC - 1)
            ed_op(BC - 1)
            # rescale chunks 4-7
            scalar.wait_ge(dma_g, 16 * (NCH + 1))
            for cc in range(4, NCH):
                op = nc.scalar.mul(o4[:, cc * CH:(cc + 1) * CH],
                                   ex4[:, cc * CH:(cc + 1) * CH],
                                   al4[0:BC, cc:cc + 1])
            op.then_inc(act_fin, 1)

        # --- DVE: P01 mul, Q tree, chunk max, tail combine, rescale ---
        @blk.vector
        def _(vector):
            def mx_op(j):
                r = j % NEB
                vector.wait_ge(pe_red, j + 1)
                nc.vector.tensor_reduce(
                    nm0[0:1, j:j + 1], ps_e[r][0:1, 0:CH],
                    mybir.AxisListType.X, mybir.AluOpType.max,
                    negate=True).then_inc(dve_mx, 1)

            n_tc = 0

            def mnb_op(b):
                vector.wait_ge(dve_mx, NCH * (b + 1))  # own nm0 writes acked
                nc.vector.tensor_reduce(
                    mnb[0:1, b:b + 1], nm0[0:1, NCH * b:NCH * (b + 1)],
                    mybir.AxisListType.X, mybir.AluOpType.min) \
                    .then_inc(dve_tl, 1)

            def wz_op(b):
                nonlocal n_tc
                vector.wait_ge(act_ed, b + 1)
                vector.wait_ge(act_exp, NCH * (b + 1))
                nc.vector.tensor_mul(w0[0:1, NCH * b:NCH * (b + 1)],
                                     ed0[0:1, NCH * b:NCH * (b + 1)],
                                     sm0[0:1, NCH * b:NCH * (b + 1)]) \
                    .then_inc(dve_tc, 1)
                n_tc += 1
                vector.wait_ge(dve_tc, n_tc)
                nc.vector.tensor_reduce(
                    zb[0:1, b:b + 1], w0[0:1, NCH * b:NCH * (b + 1)],
                    mybir.AxisListType.X, mybir.AluOpType.add) \
                    .then_inc(dve_tc, 1)
                n_tc += 1

            n_sd = 0
            vector.wait_ge(dma_h, 32)            # ht, of
            for bc in range(NBC):
                b, c, par, slot = bc // NCH, bc % NCH, bc % 2, bc % NSLOT
                # P01 = Y01 * E01
                vector.wait_ge(act_y, 2 * bc + 2)
                if bc >= 2:
                    vector.wait_ge(pool_q, bc - 1)   # WAR p_sb[par]
                nc.vector.tensor_mul(
                    p_sb[par].rearrange("p (k s) -> p k s", k=NK)[:, 0:2, :],
                    y_sb[par].rearrange("p (k s) -> p k s", k=2)[:, :, :],
                    enc_sb[slot].rearrange("p (k w) -> p k w", k=NK)
                    [:, 0:2, 1:CW]).then_inc(dve_p, 1)
                # P2, P3 stt folds (psum)
                for kt in (2, 3):
                    vector.wait_ge(pe_mm, 4 * bc + kt + 1)
                    if c == 0:
                        nc.vector.tensor_copy(
                            ps_a[:, kt * CH:kt * CH + 1], of_sb[:]) \
                            .then_inc(dve_sd, 1)
                        n_sd += 1
                        vector.wait_ge(dve_sd, n_sd)
                    nc.vector.scalar_tensor_tensor(
                        p_sb[par][:, kt * CH:(kt + 1) * CH],
                        ps_a[:, kt * CH:(kt + 1) * CH],
                        ht_sb[:, NK * b + kt:NK * b + kt + 1],
                        enc_sb[slot][:, kt * CW + 1:kt * CW + CW],
                        mybir.AluOpType.mult, mybir.AluOpType.mult) \
                        .then_inc(dve_s, 1)
                if bc >= NBC - 2:
                    # endgame: no later MM block hides the pool Q latency, so
                    # DVE computes the last two Q's itself (fp16 2x ops)
                    g = bc - (NBC - 2)
                    vector.wait_ge(dve_p, bc + 1)        # own P01 acked
                    nc.vector.tensor_add(q_sb[bc % 3][:, 0:CH],
                                         p_sb[par][:, 0:CH],
                                         p_sb[par][:, CH:2 * CH]) \
                        .then_inc(dve_qt, 1)
                    vector.wait_ge(dve_s, 2 * bc + 2)    # own stt acked
                    nc.vector.tensor_add(q_sb[bc % 3][:, CH:2 * CH],
                                         p_sb[par][:, 2 * CH:3 * CH],
                                         p_sb[par][:, 3 * CH:4 * CH]) \
                        .then_inc(dve_qt, 1)
                    vector.wait_ge(dve_qt, 2 * g + 2)
                    nc.vector.tensor_add(q_sb[bc % 3][:, 2 * CH:3 * CH],
                                         q_sb[bc % 3][:, 0:CH],
                                         q_sb[bc % 3][:, CH:2 * CH]) \
                        .then_inc(dve_q2, 1)
                if bc >= 2:
                    mx_op(bc - 2)
                if bc % NCH == 2 and bc > NCH:
                    mnb_op(bc // NCH - 1)
                if bc % NCH == 3 and bc > NCH:
                    wz_op(bc // NCH - 1)
            mx_op(NBC - 2)
            mx_op(NBC - 1)
            mnb_op(BC - 1)
            wz_op(BC - 1)
            vector.wait_ge(dve_tc, n_tc)             # zb writes acked
            nc.vector.reciprocal(rz[0:1, 0:BC], zb[0:1, 0:BC]) \
                .then_inc(dve_tc, 1)
            n_tc += 1
            vector.wait_ge(dve_tc, n_tc)             # rz write acked
            for b in range(BC):
                op = nc.vector.tensor_scalar_mul(
                    aw[32 * b:32 * b + 1, 0:NCH],
                    ed0[0:1, NCH * b:NCH * (b + 1)], rz[0:1, b:b + 1])
            op.then_inc(dve_al, 1)
            # rescale chunks 0-3
            vector.wait_ge(dma_g, 16 * (NCH + 1))
            for cc in range(4):
                op = nc.vector.tensor_scalar_mul(
                    o4[:, cc * CH:(cc + 1) * CH],
                    ex4[:, cc * CH:(cc + 1) * CH], al4[0:BC, cc:cc + 1])
            op.then_inc(dve_fin, 1)

        # --- POOL (gpsimd): P2,P3 stt folds, rescale chunks 6,7 ---
        @blk.gpsimd
        def _(gpsimd):
            for bc in range(NBC - 2):
                par, qar = bc % 2, bc % 3
                if bc >= 3:
                    gpsimd.wait_ge(pe_red, bc - 2)   # WAR q_sb[qar] (Q slice)
                gpsimd.wait_ge(dve_p, bc + 1)        # P01 landed
                nc.gpsimd.tensor_add(q_sb[qar][:, 0:CH],
                                     p_sb[par][:, 0:CH],
                                     p_sb[par][:, CH:2 * CH]) \
                    .then_inc(pool_t2, 1)
                gpsimd.wait_ge(dve_s, 2 * bc + 2)    # P2, P3 landed
                nc.gpsimd.tensor_add(q_sb[qar][:, CH:2 * CH],
                                     p_sb[par][:, 2 * CH:3 * CH],
                                     p_sb[par][:, 3 * CH:4 * CH]) \
                    .then_inc(pool_t2, 1)
                gpsimd.wait_ge(pool_t2, 2 * bc + 2)  # own writes acked
                nc.gpsimd.tensor_add(q_sb[qar][:, 2 * CH:3 * CH],
                                     q_sb[qar][:, 0:CH],
                                     q_sb[qar][:, CH:2 * CH]) \
                    .then_inc(pool_q, 1)

    return nc


def _shard_host(hidden, encoder_outputs, embedding, bigram_matrix, affect_matrix):
    """Per-core input maps. Layout/cast prep only (plus tiny h@affect)."""
    h = np.asarray(hidden, dtype=np.float32)[0]              # [B, H]
    enc = np.asarray(encoder_outputs, dtype=np.float32)      # [S, B, H]
    emb = np.asarray(embedding, dtype=np.float32)            # [S, B, 3]
    m = np.asarray(bigram_matrix, dtype=np.float32)
    aff = np.asarray(affect_matrix, dtype=np.float32)        # [H, 3]

    # padded fp16 enc: row 0 is the s=-1 halo for c==0 (value irrelevant;
    # psum col 0 is re-seeded on device)
    encp = np.zeros((S + 1, B, H), dtype=np.float16)
    encp[1:] = enc.astype(np.float16)

    m16 = m.astype(np.float16)
    m_p = np.ascontiguousarray(
        m16.reshape(NK, 128, H).transpose(1, 0, 2).reshape(128, NK * H))
    ha = (h @ aff).T.astype(np.float16)                      # [3, B]
    emb16 = np.ascontiguousarray(
        emb.transpose(1, 2, 0).astype(np.float16))           # [B, 3, S]
    one_h = np.ones((128, 1), dtype=np.float16)
    one_f = np.ones((128, 1), dtype=np.float32)

    in_maps = []
    for co in range(NCORES):
        b0 = co * BC
        # enc_c[b*NCH+c, p, k*CW+w] = encp[c*CH+w, b0+b, k*128+p]
        blocks = []
        for b in range(b0, b0 + BC):
            v = np.ascontiguousarray(encp[:, b, :])          # [S+1, H]
            w = np.lib.stride_tricks.as_strided(
                v, shape=(NCH, CW, H),
                strides=(CH * v.strides[0], v.strides[0], v.strides[1]))
            t = w.transpose(0, 2, 1).reshape(NCH, NK, 128, CW)
            blocks.append(t.transpose(0, 2, 1, 3).reshape(NCH, 128, NK * CW))
        enc_cc = np.ascontiguousarray(np.concatenate(blocks, axis=0))
        h_sl = h[b0:b0 + BC]                                 # [BC, H]
        ht = np.ascontiguousarray(
            h_sl.reshape(BC, NK, 128).transpose(2, 0, 1).reshape(128, BC * NK))
        in_maps.append({
            "enc_c": enc_cc,
            "m_p": m_p,
            "h_t": ht,
            "ha_p": np.ascontiguousarray(ha[:, b0:b0 + BC]),
            "one_h": one_h,
            "one_f": one_f,
            "emb_a": emb16[b0:b0 + BC].reshape(3 * BC, S),
        })
    return in_maps


def kernel(hidden, encoder_outputs, embedding, bigram_matrix, affect_matrix,
           _want_results=False, _spmd_kwargs=None):
    nc = _build()
    in_maps = _shard_host(hidden, encoder_outputs, embedding,
                          bigram_matrix, affect_matrix)
    res = run_bass_kernel_spmd(nc, in_maps, core_ids=list(range(NCORES)),
                               **(_spmd_kwargs or {}))
    outp = np.empty((B, 1, S), dtype=np.float32)
    for co in range(NCORES):
        outp[co * BC:(co + 1) * BC, 0, :] = res.results[co]["out"]
    if _want_results:
        return outp, res
    return outp
